# revision 1
# baseline (speedup 1.0000x reference)
"""MLA (DeepSeek-style multi-head latent attention) Bass kernel for 8 trn2 NeuronCores.

Sharding: tensor-parallel over heads (2 heads/core) for the big projections +
attention; the low-rank A-projections are sequence-sharded (256 rows/core) and
the normalized latents are AllGathered in transposed [c, s] layout. The output
projection is column-parallel (each core produces 256 output channels for all
tokens) so the final combine is a host-side concat instead of an AllReduce.

All matmuls run as float32r (full fp32 storage, PE rounded mode, 1 cyc/row at
N>=256). Softmax skips max-subtraction (scores are O(+-10), exp is safe in
fp32) so the softmax denominator is a ones-matmul partition reduction.

Host-side (free) preprocessing: all weight transposes/permutations, folding
q_norm_w/kv_norm_w and SOFTMAX_SCALE into wq_b/wkv_b, rope sign folding.
"""

import math
import sys

import numpy as np

for _p in ("/opt/trn_rl_repo", "/root/.axon_site/_ro/trn_rl_repo"):
    if _p not in sys.path:
        sys.path.append(_p)

B, S, H = 1, 2048, 2048
NH = 16
Q_LORA, KV_LORA = 1536, 512
D_NOPE, D_ROPE, D_V = 128, 64, 128
D_QK = D_NOPE + D_ROPE
ROPE_FACTOR, MSCALE = 4.0, 1.0
SOFTMAX_SCALE = D_QK ** -0.5 * (0.1 * MSCALE * math.log(ROPE_FACTOR) + 1.0) ** 2
EPS = 1e-6

NCORES = 8
SSH = S // NCORES          # 256 tokens per core in stage 0
CTOT = Q_LORA + KV_LORA + D_ROPE   # 2112 latent channels
NCT = 17                   # ceil(2112/128); tile 16 only has 64 live rows

_CACHE = {}


def _build(has_mask: bool):
    import concourse.bacc as bacc
    import concourse.mybir as mybir
    import concourse.tile as tile

    f32 = mybir.dt.float32
    f32r = mybir.dt.float32r
    AF = mybir.ActivationFunctionType
    OP = mybir.AluOpType

    nc = bacc.Bacc("TRN2", target_bir_lowering=False, debug=False,
                   num_devices=NCORES)

    hidT = nc.dram_tensor("hidT", [16, 128, SSH], f32r, kind="ExternalInput")
    a_t = nc.dram_tensor("a_t", [16, 128, CTOT], f32r, kind="ExternalInput")
    cosT_sh = nc.dram_tensor("cosT_sh", [64, SSH], f32, kind="ExternalInput")
    sinTs_sh = nc.dram_tensor("sinTs_sh", [64, SSH], f32, kind="ExternalInput")
    cosT2 = nc.dram_tensor("cosT2", [128, S], f32, kind="ExternalInput")
    sinT2s = nc.dram_tensor("sinT2s", [128, S], f32, kind="ExternalInput")
    wqbT = nc.dram_tensor("wqbT", [12, 128, 384], f32r, kind="ExternalInput")
    wkvbT = nc.dram_tensor("wkvbT", [4, 128, 512], f32r, kind="ExternalInput")
    woT = nc.dram_tensor("woT", [16, 128, SSH], f32r, kind="ExternalInput")
    ones_a = nc.dram_tensor("ones_a", [128, 1], f32r, kind="ExternalInput")
    ones_b = nc.dram_tensor("ones_b", [1, 128], f32r, kind="ExternalInput")
    zer64 = nc.dram_tensor("zer64", [64, SSH], f32r, kind="ExternalInput")
    if has_mask:
        maskT = nc.dram_tensor("maskT", [S, S], f32, kind="ExternalInput")
    out = nc.dram_tensor("out", [S, SSH], f32, kind="ExternalOutput")

    bounce1 = nc.dram_tensor("bounce1", [NCT, 128, SSH], f32r)
    gath1 = nc.dram_tensor("gath1", [NCORES, NCT, 128, SSH], f32r,
                           addr_space="Shared")
    bounce2 = nc.dram_tensor("bounce2", [2, 128, S], f32r)
    gath2 = nc.dram_tensor("gath2", [16, 128, S], f32r, addr_space="Shared")

    RG = [list(range(NCORES))]

    def mm(ps, lhsT, rhs, start, stop):
        nc.tensor.matmul(ps, lhsT, rhs, start=start, stop=stop)

    from contextlib import ExitStack
    with tile.TileContext(nc) as tc, ExitStack() as _st:
        constp = _st.enter_context(tc.tile_pool(name="const", bufs=1))
        ones_col = constp.tile([128, 1], f32r)
        nc.sync.dma_start(ones_col[:], ones_a.ap())
        ones_row = constp.tile([1, 128], f32r)
        nc.sync.dma_start(ones_row[:], ones_b.ap())
        eps_sb = constp.tile([1, 1], f32)
        nc.any.memset(eps_sb[:], EPS)

        # ---------------- stage 0: latents for own 256 tokens, [c, s] layout
        with tc.tile_pool(name="s0", bufs=1) as s0p, \
             tc.tile_pool(name="s0ps", bufs=3, space="PSUM") as s0ps, \
             tc.tile_pool(name="s0ss", bufs=1, space="PSUM") as s0ssp, \
             tc.tile_pool(name="s0pb", bufs=1, space="PSUM") as s0pb, \
             tc.tile_pool(name="s0sq", bufs=3) as s0sqp:
            hid_sb = s0p.tile([128, 16, SSH], f32r)
            nc.sync.dma_start(hid_sb[:], hidT.ap().rearrange("o p s -> p o s"))
            a_sb = s0p.tile([128, 16, CTOT], f32r)
            for c0, cw in ((0, 512), (512, 512), (1024, 512), (1536, 576)):
                nc.sync.dma_start(
                    a_sb[:, :, c0:c0 + cw],
                    a_t.ap()[:, :, c0:c0 + cw].rearrange("o p c -> p o c"))

            raw = s0p.tile([128, NCT, SSH], f32)
            ss_hq = s0ssp.tile([1, SSH], f32)
            ss_kv = s0ssp.tile([1, SSH], f32)
            for ct in range(NCT):
                w = 128 if ct < 16 else 64
                ps = s0ps.tile([128, SSH], f32, tag="s0ps")
                for hb in range(16):
                    mm(ps[:w], a_sb[:, hb, ct * 128:ct * 128 + w],
                       hid_sb[:, hb, :], hb == 0, hb == 15)
                nc.vector.tensor_copy(raw[:w, ct, :], ps[:w])
                if ct < 16:
                    sq = s0sqp.tile([128, SSH], f32r, tag="s0sq")
                    nc.scalar.activation(sq[:], ps[:], AF.Square)
                    if ct < 12:
                        mm(ss_hq, ones_col, sq, ct == 0, ct == 11)
                    else:
                        mm(ss_kv, ones_col, sq, ct == 12, ct == 15)

            # rms scale factors: rsqrt(sumsq/D + eps), broadcast to 128 parts
            sq_hq = s0p.tile([1, SSH], f32)
            nc.scalar.activation(sq_hq[:], ss_hq[:], AF.Sqrt,
                                 bias=eps_sb[:], scale=1.0 / Q_LORA)
            rc_hq = s0p.tile([1, SSH], f32r)
            with nc.allow_low_precision(reason="f32r rms scale is fine"):
                nc.vector.reciprocal(rc_hq[:], sq_hq[:])
            sq_kv = s0p.tile([1, SSH], f32)
            nc.scalar.activation(sq_kv[:], ss_kv[:], AF.Sqrt,
                                 bias=eps_sb[:], scale=1.0 / KV_LORA)
            rc_kv = s0p.tile([1, SSH], f32r)
            with nc.allow_low_precision(reason="f32r rms scale is fine"):
                nc.vector.reciprocal(rc_kv[:], sq_kv[:])

            psb_hq = s0pb.tile([128, SSH], f32, tag="s0pb")
            mm(psb_hq, ones_row, rc_hq, True, True)
            bc_hq = s0p.tile([128, SSH], f32)
            nc.scalar.copy(bc_hq[:], psb_hq[:])
            psb_kv = s0pb.tile([128, SSH], f32, tag="s0pb")
            mm(psb_kv, ones_row, rc_kv, True, True)
            bc_kv = s0p.tile([128, SSH], f32)
            nc.scalar.copy(bc_kv[:], psb_kv[:])

            lat = s0p.tile([128, NCT, SSH], f32r)
            for ct in range(12):
                nc.vector.tensor_tensor(lat[:, ct, :], raw[:, ct, :],
                                        bc_hq[:], OP.mult)
            for ct in range(12, 16):
                nc.vector.tensor_tensor(lat[:, ct, :], raw[:, ct, :],
                                        bc_kv[:], OP.mult)
            # k_pe rope (not normalized); rows [0:64) of c-tile 16
            cs_sb = s0p.tile([64, SSH], f32)
            nc.sync.dma_start(cs_sb[:], cosT_sh.ap())
            sn_sb = s0p.tile([64, SSH], f32)
            nc.sync.dma_start(sn_sb[:], sinTs_sh.ap())
            t1 = s0p.tile([64, SSH], f32)
            nc.vector.tensor_tensor(t1[:], raw[0:64, 16, :], cs_sb[:], OP.mult)
            rsw = s0p.tile([64, SSH], f32)
            nc.sync.dma_start(rsw[0:32], raw[32:64, 16, :])
            nc.sync.dma_start(rsw[32:64], raw[0:32, 16, :])
            t2 = s0p.tile([64, SSH], f32)
            nc.vector.tensor_tensor(t2[:], rsw[:], sn_sb[:], OP.mult)
            nc.vector.tensor_tensor(lat[0:64, 16, :], t1[:], t2[:], OP.add)
            nc.sync.dma_start(lat[64:128, 16, :], zer64.ap())
            nc.sync.dma_start(bounce1.ap().rearrange("o p s -> p o s"), lat[:])

        nc.gpsimd.collective_compute(
            "AllGather", OP.bypass, replica_groups=RG,
            ins=[bounce1.ap().opt()], outs=[gath1.ap().opt()])

        # ---------------- stage 1: per-head projections + attention
        with tc.tile_pool(name="s1w", bufs=1) as s1w, \
             tc.tile_pool(name="att", bufs=1) as attp:
            wqb_sb = s1w.tile([128, 12, 384], f32r)
            nc.sync.dma_start(wqb_sb[:], wqbT.ap().rearrange("o p d -> p o d"))
            wkvb_sb = s1w.tile([128, 4, 512], f32r)
            nc.sync.dma_start(wkvb_sb[:], wkvbT.ap().rearrange("o p d -> p o d"))

            kv_sb = s1w.tile([128, 32, SSH], f32r)
            kpe_sb = attp.tile([64, 8, SSH], f32r)
            for r in range(NCORES):
                nc.sync.dma_start(
                    kv_sb[:, r * 4:(r + 1) * 4, :],
                    gath1.ap()[r, 12:16].rearrange("o p s -> p o s"))
                nc.sync.dma_start(kpe_sb[:, r, :], gath1.ap()[r, 16, 0:64, :])

            qn0 = attp.tile([128, S], f32r)
            qt1 = attp.tile([128, S], f32)
            qn1 = attp.tile([128, S], f32r)
            qdst = (qn0, qt1, qn1)
            kn0 = attp.tile([128, S], f32r)
            kn1 = attp.tile([128, S], f32r)
            kn = (kn0, kn1)
            vt = [attp.tile([128, 256], f32r, name=f"vt{tb}")
                  for tb in range(16)]

            with tc.tile_pool(name="hq", bufs=2) as hqp, \
                 tc.tile_pool(name="p1ps", bufs=3, space="PSUM") as p1ps:
                for r in range(NCORES):
                    hq_sb = hqp.tile([128, 12, SSH], f32r, tag="hq")
                    nc.sync.dma_start(
                        hq_sb[:], gath1.ap()[r, 0:12].rearrange("o p s -> p o s"))
                    for m in range(3):
                        ps = p1ps.tile([128, SSH], f32, tag="p1ps")
                        for cc in range(12):
                            mm(ps, wqb_sb[:, cc, m * 128:(m + 1) * 128],
                               hq_sb[:, cc, :], cc == 0, cc == 11)
                        nc.scalar.copy(qdst[m][:, r * SSH:(r + 1) * SSH], ps[:])
                for kh in range(2):
                    for t8 in range(8):
                        ps = p1ps.tile([128, SSH], f32, tag="p1ps")
                        for cc in range(4):
                            mm(ps, wkvb_sb[:, cc, kh * 128:(kh + 1) * 128],
                               kv_sb[:, t8 * 4 + cc, :], cc == 0, cc == 3)
                        nc.scalar.copy(kn[kh][:, t8 * SSH:(t8 + 1) * SSH], ps[:])
                for tb in range(16):
                    ps = p1ps.tile([128, SSH], f32, tag="p1ps")
                    for cc in range(4):
                        mm(ps, kv_sb[:, (tb // 2) * 4 + cc,
                                     (tb % 2) * 128:(tb % 2) * 128 + 128],
                           wkvb_sb[:, cc, 256:512], cc == 0, cc == 3)
                        # lhsT = kvnT chunk [c,t], rhs = v columns of wkv_b'^T
                    nc.scalar.copy(vt[tb][:], ps[:])

            # rope on q (both heads share qt1: rows 0:64 h0, 64:128 h1)
            qt1r = attp.tile([128, S], f32r)
            qr1 = attp.tile([64, S], f32r)
            with tc.tile_pool(name="rope", bufs=1) as rp:
                cos2_sb = rp.tile([128, S], f32)
                nc.sync.dma_start(cos2_sb[:], cosT2.ap())
                sin2_sb = rp.tile([128, S], f32)
                nc.sync.dma_start(sin2_sb[:], sinT2s.ap())
                tmp = rp.tile([128, S], f32)
                for b in (0, 64):
                    nc.sync.dma_start(tmp[b:b + 32], qt1[b + 32:b + 64])
                    nc.sync.dma_start(tmp[b + 32:b + 64], qt1[b:b + 32])
                nc.vector.tensor_tensor(qt1r[:], qt1[:], cos2_sb[:], OP.mult)
                nc.vector.tensor_tensor(tmp[:], tmp[:], sin2_sb[:], OP.mult)
                nc.vector.tensor_tensor(qt1r[:], qt1r[:], tmp[:], OP.add)
                # h1 rope rows to a base-0 tile for use as matmul rhs
                nc.sync.dma_start(qr1[:], qt1r[64:128])

            # attention, streaming over t in chunks of 128
            with tc.tile_pool(name="apss", bufs=2, space="PSUM") as apss, \
                 tc.tile_pool(name="apsx", bufs=2, space="PSUM") as apsx, \
                 tc.tile_pool(name="apsd", bufs=2, space="PSUM") as apsd, \
                 tc.tile_pool(name="apsb", bufs=2, space="PSUM") as apsb, \
                 tc.tile_pool(name="aex", bufs=3) as aexp, \
                 tc.tile_pool(name="asm", bufs=2) as asmp, \
                 tc.tile_pool(name="amk", bufs=2) as amkp, \
                 tc.tile_pool(name="xh", bufs=1) as xhp:
                for h in range(2):
                    qr_h = qt1r if h == 0 else qr1
                    xh = xhp.tile([128, S], f32r, name=f"xh{h}")
                    for sb in range(4):
                        psx = apsx.tile([128, 512], f32, tag="apsx")
                        psd = apsd.tile([1, 512], f32, tag="apsd")
                        for tb in range(16):
                            pss = apss.tile([128, 512], f32, tag="apss")
                            mm(pss, kn[h][:, tb * 128:(tb + 1) * 128],
                               qn0[:, sb * 512:(sb + 1) * 512] if h == 0
                               else qn1[:, sb * 512:(sb + 1) * 512],
                               True, False)
                            mm(pss, kpe_sb[:, tb // 2,
                                           (tb % 2) * 128:(tb % 2) * 128 + 128],
                               qr_h[0:64, sb * 512:(sb + 1) * 512],
                               False, True)
                            if has_mask:
                                mk = amkp.tile([128, 512], f32, tag="amk")
                                nc.sync.dma_start(
                                    mk[:], maskT.ap()[tb * 128:(tb + 1) * 128,
                                                      sb * 512:(sb + 1) * 512])
                                nc.vector.tensor_tensor(pss[:], pss[:], mk[:],
                                                        OP.add)
                            ex = aexp.tile([128, 512], f32r, tag="aex")
                            nc.scalar.activation(ex[:], pss[:], AF.Exp)
                            mm(psx, vt[tb][:, h * 128:(h + 1) * 128], ex,
                               tb == 0, tb == 15)
                            mm(psd, ones_col, ex, tb == 0, tb == 15)
                        rd = asmp.tile([1, 512], f32r, tag="rd")
                        with nc.allow_low_precision(reason="f32r softmax denom"):
                            nc.vector.reciprocal(rd[:], psd[:])
                        psb2 = apsb.tile([128, 512], f32, tag="apsb")
                        mm(psb2, ones_row, rd, True, True)
                        rdb = asmp.tile([128, 512], f32, tag="rdb")
                        nc.vector.tensor_copy(rdb[:], psb2[:])
                        nc.vector.tensor_tensor(
                            xh[:, sb * 512:(sb + 1) * 512], psx[:], rdb[:],
                            OP.mult)
                    nc.sync.dma_start(bounce2.ap()[h], xh[:])

        nc.gpsimd.collective_compute(
            "AllGather", OP.bypass, replica_groups=RG,
            ins=[bounce2.ap().opt()], outs=[gath2.ap().opt()])

        # ---------------- output projection (column-parallel over H)
        with tc.tile_pool(name="wo", bufs=1) as wop, \
             tc.tile_pool(name="wops", bufs=2, space="PSUM") as wops, \
             tc.tile_pool(name="woot", bufs=3) as wootp:
            wot_sb = wop.tile([128, 16, SSH], f32r)
            nc.sync.dma_start(wot_sb[:], woT.ap().rearrange("o p s -> p o s"))
            big_xe = wop.tile([128, 16, S], f32r)
            for k in range(16):
                nc.sync.dma_start(big_xe[:, k, :], gath2.ap()[k])
            for st in range(16):
                pso = wops.tile([128, SSH], f32, tag="wops")
                for k in range(16):
                    mm(pso, big_xe[:, k, st * 128:(st + 1) * 128],
                       wot_sb[:, k, :], k == 0, k == 15)
                ot = wootp.tile([128, SSH], f32, tag="ot")
                nc.scalar.copy(ot[:], pso[:])
                nc.sync.dma_start(out.ap()[st * 128:(st + 1) * 128, :], ot[:])

    nc.compile()
    return nc


def _prep_inputs(hidden_states, cos, sin, attn_mask, wq_a, q_norm_w, wq_b,
                 wkv_a, kv_norm_w, wkv_b, wo, has_mask):
    c = np.ascontiguousarray
    hid = np.asarray(hidden_states, np.float32)[0]          # [S, H]
    hidT = hid.T                                            # [H, S]
    A_T = np.vstack([np.asarray(wq_a, np.float32),
                     np.asarray(wkv_a, np.float32)]).T      # [H, CTOT]
    a_t = c(A_T.reshape(16, 128, CTOT))

    cosT = np.asarray(cos, np.float32).T                    # [64, S]
    sinT = np.asarray(sin, np.float32).T
    sinTs = sinT.copy()
    sinTs[0:32] *= -1.0
    cosT2 = c(np.concatenate([cosT, cosT], 0))              # [128, S]
    sinT2s = c(np.concatenate([sinTs, sinTs], 0))

    wqb = np.asarray(wq_b, np.float32) * np.asarray(q_norm_w, np.float32)[None]
    wqb = wqb * SOFTMAX_SCALE
    wkvb = (np.asarray(wkv_b, np.float32)
            * np.asarray(kv_norm_w, np.float32)[None])
    woT_full = np.asarray(wo, np.float32).T                 # [NH*DV, H]

    qperm = np.r_[0:128, 128:192, 320:384, 192:320]
    kvperm = np.r_[0:128, 256:384, 128:256, 384:512]

    in_maps = []
    for r in range(NCORES):
        m = {
            "hidT": c(hidT[:, r * SSH:(r + 1) * SSH].reshape(16, 128, SSH)),
            "a_t": a_t,
            "cosT_sh": c(cosT[:, r * SSH:(r + 1) * SSH]),
            "sinTs_sh": c(sinTs[:, r * SSH:(r + 1) * SSH]),
            "cosT2": cosT2,
            "sinT2s": sinT2s,
            "wqbT": c(wqb[r * 384:(r + 1) * 384].T[:, qperm]
                      .reshape(12, 128, 384)),
            "wkvbT": c(wkvb[r * 512:(r + 1) * 512].T[:, kvperm]
                       .reshape(4, 128, 512)),
            "woT": c(woT_full[:, r * SSH:(r + 1) * SSH].reshape(16, 128, SSH)),
            "ones_a": np.ones((128, 1), np.float32),
            "ones_b": np.ones((1, 128), np.float32),
            "zer64": np.zeros((64, SSH), np.float32),
        }
        if has_mask:
            m["maskT"] = c(np.asarray(attn_mask, np.float32).T)
        in_maps.append(m)
    return in_maps


def kernel(**inputs):
    from concourse.bass_utils import run_bass_kernel_spmd

    has_mask = bool(np.any(np.asarray(inputs["attn_mask"])))
    if has_mask not in _CACHE:
        _CACHE[has_mask] = _build(has_mask)
    nc = _CACHE[has_mask]

    in_maps = _prep_inputs(has_mask=has_mask, **inputs)
    res = run_bass_kernel_spmd(nc, in_maps, list(range(NCORES))).results
    full = np.concatenate([res[r]["out"] for r in range(NCORES)], axis=1)
    return full.reshape(B, S, H).astype(np.float32)



# revision 5
# speedup vs baseline: 1.3242x; 1.3242x over previous
"""MLA (DeepSeek-style multi-head latent attention) Bass kernel for 8 trn2 NeuronCores.

Sharding: tensor-parallel over heads (2 heads/core) for the big projections +
attention; the low-rank A-projections are sequence-sharded (256 rows/core) and
the normalized latents are AllGathered in transposed [c, s] layout. The output
projection is column-parallel (each core produces 256 output channels for all
tokens) so the final combine is a host-side concat instead of an AllReduce.

All tensors are stored bf16 (fp32 PSUM accumulation everywhere): halves DMA
and collective bytes, doubles PE streaming rate (fp32 moves at 2 cyc/elem),
and enables fast-weight-load. RMS scale factors stay fp32. Softmax skips
max-subtraction (scores are O(+-10), exp is safe in fp32).

Host-side (free) preprocessing: bf16 casts, all weight transposes/permutations,
folding q_norm_w/kv_norm_w and SOFTMAX_SCALE into wq_b/wkv_b, rope sign folding.
"""

import math
import sys

import numpy as np

for _p in ("/opt/trn_rl_repo", "/root/.axon_site/_ro/trn_rl_repo"):
    if _p not in sys.path:
        sys.path.append(_p)

B, S, H = 1, 2048, 2048
NH = 16
Q_LORA, KV_LORA = 1536, 512
D_NOPE, D_ROPE, D_V = 128, 64, 128
D_QK = D_NOPE + D_ROPE
ROPE_FACTOR, MSCALE = 4.0, 1.0
SOFTMAX_SCALE = D_QK ** -0.5 * (0.1 * MSCALE * math.log(ROPE_FACTOR) + 1.0) ** 2
EPS = 1e-6

NCORES = 8
SSH = S // NCORES          # 256 tokens per core in stage 0
CTOT = Q_LORA + KV_LORA + D_ROPE   # 2112 latent channels
NCT = 17                   # ceil(2112/128); tile 16 only has 64 live rows

_CACHE = {}


def _build(has_mask: bool):
    import concourse.bacc as bacc
    import concourse.mybir as mybir
    import concourse.tile as tile

    f32 = mybir.dt.float32
    b16 = mybir.dt.bfloat16
    AF = mybir.ActivationFunctionType
    OP = mybir.AluOpType

    nc = bacc.Bacc("TRN2", target_bir_lowering=False, debug=False,
                   num_devices=NCORES)

    hidT = nc.dram_tensor("hidT", [16, 128, SSH], b16, kind="ExternalInput")
    a_t = nc.dram_tensor("a_t", [16, 128, CTOT], b16, kind="ExternalInput")
    cosT_sh = nc.dram_tensor("cosT_sh", [64, SSH], b16, kind="ExternalInput")
    sinTs_sh = nc.dram_tensor("sinTs_sh", [64, SSH], b16, kind="ExternalInput")
    cosT2 = nc.dram_tensor("cosT2", [128, S], b16, kind="ExternalInput")
    sinT2s = nc.dram_tensor("sinT2s", [128, S], b16, kind="ExternalInput")
    wqbT = nc.dram_tensor("wqbT", [12, 128, 384], b16, kind="ExternalInput")
    wkvbT = nc.dram_tensor("wkvbT", [4, 128, 512], b16, kind="ExternalInput")
    woT = nc.dram_tensor("woT", [16, 128, SSH], b16, kind="ExternalInput")
    ones_a = nc.dram_tensor("ones_a", [128, 1], b16, kind="ExternalInput")
    ones_bf = nc.dram_tensor("ones_bf", [1, 128], f32, kind="ExternalInput")
    zer64 = nc.dram_tensor("zer64", [64, SSH], b16, kind="ExternalInput")
    if has_mask:
        maskT = nc.dram_tensor("maskT", [S, S], b16, kind="ExternalInput")
    out = nc.dram_tensor("out", [S, SSH], f32, kind="ExternalOutput")

    bounce1 = nc.dram_tensor("bounce1", [NCT, 128, SSH], b16)
    gath1 = nc.dram_tensor("gath1", [NCORES, NCT, 128, SSH], b16,
                           addr_space="Shared")
    bounce2 = nc.dram_tensor("bounce2", [2, 128, S], b16)
    gath2 = nc.dram_tensor("gath2", [16, 128, S], b16, addr_space="Shared")

    RG = [list(range(NCORES))]

    def mm(ps, lhsT, rhs, start, stop):
        nc.tensor.matmul(ps, lhsT, rhs, start=start, stop=stop)

    from contextlib import ExitStack
    with tile.TileContext(nc) as tc, ExitStack() as _st:
        constp = _st.enter_context(tc.tile_pool(name="const", bufs=1))
        ones_col = constp.tile([128, 1], b16)
        nc.sync.dma_start(ones_col[:], ones_a.ap())
        ones_row = constp.tile([1, 128], f32)
        nc.sync.dma_start(ones_row[:], ones_bf.ap())
        eps_sb = constp.tile([1, 1], f32)
        nc.any.memset(eps_sb[:], EPS)

        # stage-1 weights prefetched up front (overlap with stage 0)
        s1wp = _st.enter_context(tc.tile_pool(name="s1w", bufs=1))
        wqb_sb = s1wp.tile([128, 12, 384], b16)
        nc.sync.dma_start(wqb_sb[:], wqbT.ap().rearrange("o p d -> p o d"))
        wkvb_sb = s1wp.tile([128, 4, 512], b16)
        nc.sync.dma_start(wkvb_sb[:], wkvbT.ap().rearrange("o p d -> p o d"))
        cos2_sb = s1wp.tile([128, S], b16)
        nc.sync.dma_start(cos2_sb[:], cosT2.ap())
        sin2_sb = s1wp.tile([128, S], b16)
        nc.sync.dma_start(sin2_sb[:], sinT2s.ap())
        # wo weights: tile allocated up-front (pool stack order), DMA issued
        # later so it doesn't compete with stage-0 loads
        wot_sb = s1wp.tile([128, 16, SSH], b16)

        # ---------------- stage 0: latents for own 256 tokens, [c, s] layout
        with tc.tile_pool(name="s0", bufs=1) as s0p, \
             tc.tile_pool(name="s0ps", bufs=3, space="PSUM") as s0ps, \
             tc.tile_pool(name="s0ss", bufs=1, space="PSUM") as s0ssp, \
             tc.tile_pool(name="s0pb", bufs=1, space="PSUM") as s0pb, \
             tc.tile_pool(name="s0sq", bufs=3) as s0sqp:
            hid_sb = s0p.tile([128, 16, SSH], b16)
            nc.sync.dma_start(hid_sb[:], hidT.ap().rearrange("o p s -> p o s"))
            a_sb = s0p.tile([128, 16, CTOT], b16)
            for c0, cw in ((0, 512), (512, 512), (1024, 512), (1536, 576)):
                nc.sync.dma_start(
                    a_sb[:, :, c0:c0 + cw],
                    a_t.ap()[:, :, c0:c0 + cw].rearrange("o p c -> p o c"))

            raw = s0p.tile([128, 16, SSH], b16)
            rawpe = s0p.tile([64, SSH], f32)
            ss_hq = s0ssp.tile([1, SSH], f32)
            ss_kv = s0ssp.tile([1, SSH], f32)
            for ct in range(NCT):
                w = 128 if ct < 16 else 64
                ps = s0ps.tile([128, SSH], f32, tag="s0ps")
                for hb in range(16):
                    mm(ps[:w], a_sb[:, hb, ct * 128:ct * 128 + w],
                       hid_sb[:, hb, :], hb == 0, hb == 15)
                if ct < 16:
                    nc.vector.tensor_copy(raw[:, ct, :], ps[:])
                    sq = s0sqp.tile([128, SSH], b16, tag="s0sq")
                    nc.scalar.activation(sq[:], ps[:], AF.Square)
                    if ct < 12:
                        mm(ss_hq, ones_col, sq, ct == 0, ct == 11)
                    else:
                        mm(ss_kv, ones_col, sq, ct == 12, ct == 15)
                else:
                    nc.vector.tensor_copy(rawpe[:], ps[:w])

            # rms scale factors: rsqrt(sumsq/D + eps), broadcast to 128 parts
            sq_hq = s0p.tile([1, SSH], f32)
            nc.scalar.activation(sq_hq[:], ss_hq[:], AF.Sqrt,
                                 bias=eps_sb[:], scale=1.0 / Q_LORA)
            rc_hq = s0p.tile([1, SSH], f32)
            nc.vector.reciprocal(rc_hq[:], sq_hq[:])
            sq_kv = s0p.tile([1, SSH], f32)
            nc.scalar.activation(sq_kv[:], ss_kv[:], AF.Sqrt,
                                 bias=eps_sb[:], scale=1.0 / KV_LORA)
            rc_kv = s0p.tile([1, SSH], f32)
            nc.vector.reciprocal(rc_kv[:], sq_kv[:])

            psb_hq = s0pb.tile([128, SSH], f32, tag="s0pb")
            mm(psb_hq, ones_row, rc_hq, True, True)
            bc_hq = s0p.tile([128, SSH], f32)
            nc.scalar.copy(bc_hq[:], psb_hq[:])
            psb_kv = s0pb.tile([128, SSH], f32, tag="s0pb")
            mm(psb_kv, ones_row, rc_kv, True, True)
            bc_kv = s0p.tile([128, SSH], f32)
            nc.scalar.copy(bc_kv[:], psb_kv[:])

            lat = s0p.tile([128, NCT, SSH], b16)
            for ct in range(12):
                nc.vector.tensor_tensor(lat[:, ct, :], raw[:, ct, :],
                                        bc_hq[:], OP.mult)
            for ct in range(12, 16):
                nc.vector.tensor_tensor(lat[:, ct, :], raw[:, ct, :],
                                        bc_kv[:], OP.mult)
            # k_pe rope (not normalized); rows [0:64) of c-tile 16
            cs_sb = s0p.tile([64, SSH], b16)
            nc.sync.dma_start(cs_sb[:], cosT_sh.ap())
            sn_sb = s0p.tile([64, SSH], b16)
            nc.sync.dma_start(sn_sb[:], sinTs_sh.ap())
            t1 = s0p.tile([64, SSH], f32)
            nc.vector.tensor_tensor(t1[:], rawpe[:], cs_sb[:], OP.mult)
            rsw = s0p.tile([64, SSH], f32)
            nc.sync.dma_start(rsw[0:32], rawpe[32:64])
            nc.sync.dma_start(rsw[32:64], rawpe[0:32])
            t2 = s0p.tile([64, SSH], f32)
            nc.vector.tensor_tensor(t2[:], rsw[:], sn_sb[:], OP.mult)
            nc.vector.tensor_tensor(lat[0:64, 16, :], t1[:], t2[:], OP.add)
            nc.sync.dma_start(lat[64:128, 16, :], zer64.ap())
            nc.sync.dma_start(bounce1.ap().rearrange("o p s -> p o s"), lat[:])

        nc.gpsimd.collective_compute(
            "AllGather", OP.bypass, replica_groups=RG,
            ins=[bounce1.ap().opt()], outs=[gath1.ap().opt()])

        # ---------------- stage 1: per-head projections + attention
        with tc.tile_pool(name="att", bufs=1) as attp:
            kv_sb = attp.tile([128, 4, S], b16)
            kpe_sb = attp.tile([64, S], b16)
            for r in range(NCORES):
                nc.sync.dma_start(
                    kv_sb[:, :, r * SSH:(r + 1) * SSH],
                    gath1.ap()[r, 12:16].rearrange("o p s -> p o s"))
                nc.sync.dma_start(kpe_sb[:, r * SSH:(r + 1) * SSH],
                                  gath1.ap()[r, 16, 0:64, :])

            qn0 = attp.tile([128, S], b16)
            qt1 = attp.tile([128, S], b16)
            qn1 = attp.tile([128, S], b16)
            qdst = (qn0, qt1, qn1)
            kn0 = attp.tile([128, S], b16)
            kn1 = attp.tile([128, S], b16)
            kn = (kn0, kn1)
            vt = [attp.tile([128, 256], b16, name=f"vt{tb}")
                  for tb in range(16)]

            # q projection: token pairs (512-wide rhs)
            with tc.tile_pool(name="hq", bufs=2) as hqp, \
                 tc.tile_pool(name="p1ps", bufs=3, space="PSUM") as p1ps:
                for tt in range(4):
                    hq_sb = hqp.tile([128, 12, 512], b16, tag="hq")
                    for half in range(2):
                        r = tt * 2 + half
                        nc.sync.dma_start(
                            hq_sb[:, :, half * SSH:(half + 1) * SSH],
                            gath1.ap()[r, 0:12].rearrange("o p s -> p o s"))
                    for m in range(3):
                        ps = p1ps.tile([128, 512], f32, tag="p1ps")
                        for cc in range(12):
                            mm(ps, wqb_sb[:, cc, m * 128:(m + 1) * 128],
                               hq_sb[:, cc, :], cc == 0, cc == 11)
                        nc.vector.tensor_copy(
                            qdst[m][:, tt * 512:(tt + 1) * 512], ps[:])
                for kh in range(2):
                    for tt in range(4):
                        ps = p1ps.tile([128, 512], f32, tag="p1ps")
                        for cc in range(4):
                            mm(ps, wkvb_sb[:, cc, kh * 128:(kh + 1) * 128],
                               kv_sb[:, cc, tt * 512:(tt + 1) * 512],
                               cc == 0, cc == 3)
                        nc.vector.tensor_copy(
                            kn[kh][:, tt * 512:(tt + 1) * 512], ps[:])
                for tb in range(16):
                    ps = p1ps.tile([128, 256], f32, tag="p1ps")
                    for cc in range(4):
                        mm(ps, kv_sb[:, cc, tb * 128:(tb + 1) * 128],
                           wkvb_sb[:, cc, 256:512], cc == 0, cc == 3)
                        # lhsT = kvnT chunk [c,t], rhs = v columns of wkv_b'^T
                    nc.vector.tensor_copy(vt[tb][:], ps[:])

            # rope on q (both heads share qt1: rows 0:64 h0, 64:128 h1)
            qt1r = attp.tile([128, S], b16)
            qr1 = attp.tile([64, S], b16)
            with tc.tile_pool(name="rope", bufs=1) as rp:
                tmp = rp.tile([128, S], b16)
                for b in (0, 64):
                    nc.sync.dma_start(tmp[b:b + 32], qt1[b + 32:b + 64])
                    nc.sync.dma_start(tmp[b + 32:b + 64], qt1[b:b + 32])
                nc.vector.tensor_tensor(qt1r[:], qt1[:], cos2_sb[:], OP.mult)
                nc.vector.tensor_tensor(tmp[:], tmp[:], sin2_sb[:], OP.mult)
                nc.vector.tensor_tensor(qt1r[:], qt1r[:], tmp[:], OP.add)
                # h1 rope rows to a base-0 tile for use as matmul rhs
                nc.sync.dma_start(qr1[:], qt1r[64:128])

            # wo weights prefetch (overlaps attention)
            nc.sync.dma_start(wot_sb[:], woT.ap().rearrange("o p s -> p o s"))

            # attention, streaming over t in chunks of 128
            with tc.tile_pool(name="apss", bufs=2, space="PSUM") as apss, \
                 tc.tile_pool(name="apsx", bufs=2, space="PSUM") as apsx, \
                 tc.tile_pool(name="apsd", bufs=2, space="PSUM") as apsd, \
                 tc.tile_pool(name="apsb", bufs=2, space="PSUM") as apsb, \
                 tc.tile_pool(name="aex", bufs=3) as aexp, \
                 tc.tile_pool(name="asm", bufs=2) as asmp, \
                 tc.tile_pool(name="amk", bufs=2) as amkp, \
                 tc.tile_pool(name="xh", bufs=1) as xhp:
                for h in range(2):
                    qr_h = qt1r if h == 0 else qr1
                    xh = xhp.tile([128, S], b16, name=f"xh{h}")
                    for sb in range(4):
                        psx = apsx.tile([128, 512], f32, tag="apsx")
                        psd = apsd.tile([1, 512], f32, tag="apsd")
                        for tb in range(16):
                            pss = apss.tile([128, 512], f32, tag="apss")
                            mm(pss, kn[h][:, tb * 128:(tb + 1) * 128],
                               qn0[:, sb * 512:(sb + 1) * 512] if h == 0
                               else qn1[:, sb * 512:(sb + 1) * 512],
                               True, False)
                            mm(pss, kpe_sb[:, tb * 128:(tb + 1) * 128],
                               qr_h[0:64, sb * 512:(sb + 1) * 512],
                               False, True)
                            if has_mask:
                                mk = amkp.tile([128, 512], b16, tag="amk")
                                nc.sync.dma_start(
                                    mk[:], maskT.ap()[tb * 128:(tb + 1) * 128,
                                                      sb * 512:(sb + 1) * 512])
                                nc.vector.tensor_tensor(pss[:], pss[:], mk[:],
                                                        OP.add)
                            ex = aexp.tile([128, 512], b16, tag="aex")
                            nc.scalar.activation(ex[:], pss[:], AF.Exp)
                            mm(psx, vt[tb][:, h * 128:(h + 1) * 128], ex,
                               tb == 0, tb == 15)
                            mm(psd, ones_col, ex, tb == 0, tb == 15)
                        rd = asmp.tile([1, 512], f32, tag="rd")
                        nc.vector.reciprocal(rd[:], psd[:])
                        psb2 = apsb.tile([128, 512], f32, tag="apsb")
                        mm(psb2, ones_row, rd, True, True)
                        rdb = asmp.tile([128, 512], f32, tag="rdb")
                        nc.vector.tensor_copy(rdb[:], psb2[:])
                        nc.vector.tensor_tensor(
                            xh[:, sb * 512:(sb + 1) * 512], psx[:], rdb[:],
                            OP.mult)
                    nc.sync.dma_start(bounce2.ap()[h], xh[:])

        nc.gpsimd.collective_compute(
            "AllGather", OP.bypass, replica_groups=RG,
            ins=[bounce2.ap().opt()], outs=[gath2.ap().opt()])

        # ---------------- output projection (column-parallel over H)
        with tc.tile_pool(name="wops", bufs=2, space="PSUM") as wops, \
             tc.tile_pool(name="woot", bufs=3) as wootp, \
             tc.tile_pool(name="woxe", bufs=1) as woxep:
            big_xe = woxep.tile([128, 16, S], b16)
            for k in range(16):
                nc.sync.dma_start(big_xe[:, k, :], gath2.ap()[k])
            for st in range(16):
                pso = wops.tile([128, SSH], f32, tag="wops")
                for k in range(16):
                    mm(pso, big_xe[:, k, st * 128:(st + 1) * 128],
                       wot_sb[:, k, :], k == 0, k == 15)
                ot = wootp.tile([128, SSH], f32, tag="ot")
                nc.scalar.copy(ot[:], pso[:])
                nc.sync.dma_start(out.ap()[st * 128:(st + 1) * 128, :], ot[:])

    nc.compile()
    return nc


def _prep_inputs(hidden_states, cos, sin, attn_mask, wq_a, q_norm_w, wq_b,
                 wkv_a, kv_norm_w, wkv_b, wo, has_mask):
    import ml_dtypes
    bf = ml_dtypes.bfloat16

    def c(x):
        return np.ascontiguousarray(x.astype(bf))

    hid = np.asarray(hidden_states, np.float32)[0]          # [S, H]
    hidT = hid.T                                            # [H, S]
    A_T = np.vstack([np.asarray(wq_a, np.float32),
                     np.asarray(wkv_a, np.float32)]).T      # [H, CTOT]
    a_t = c(A_T.reshape(16, 128, CTOT))

    cosT = np.asarray(cos, np.float32).T                    # [64, S]
    sinT = np.asarray(sin, np.float32).T
    sinTs = sinT.copy()
    sinTs[0:32] *= -1.0
    cosT2 = c(np.concatenate([cosT, cosT], 0))              # [128, S]
    sinT2s = c(np.concatenate([sinTs, sinTs], 0))

    wqb = np.asarray(wq_b, np.float32) * np.asarray(q_norm_w, np.float32)[None]
    wqb = wqb * SOFTMAX_SCALE
    wkvb = (np.asarray(wkv_b, np.float32)
            * np.asarray(kv_norm_w, np.float32)[None])
    woT_full = np.asarray(wo, np.float32).T                 # [NH*DV, H]

    qperm = np.r_[0:128, 128:192, 320:384, 192:320]
    kvperm = np.r_[0:128, 256:384, 128:256, 384:512]

    in_maps = []
    for r in range(NCORES):
        m = {
            "hidT": c(hidT[:, r * SSH:(r + 1) * SSH].reshape(16, 128, SSH)),
            "a_t": a_t,
            "cosT_sh": c(cosT[:, r * SSH:(r + 1) * SSH]),
            "sinTs_sh": c(sinTs[:, r * SSH:(r + 1) * SSH]),
            "cosT2": cosT2,
            "sinT2s": sinT2s,
            "wqbT": c(wqb[r * 384:(r + 1) * 384].T[:, qperm]
                      .reshape(12, 128, 384)),
            "wkvbT": c(wkvb[r * 512:(r + 1) * 512].T[:, kvperm]
                       .reshape(4, 128, 512)),
            "woT": c(woT_full[:, r * SSH:(r + 1) * SSH].reshape(16, 128, SSH)),
            "ones_a": np.ones((128, 1), bf),
            "ones_bf": np.ones((1, 128), np.float32),
            "zer64": np.zeros((64, SSH), bf),
        }
        if has_mask:
            m["maskT"] = c(np.asarray(attn_mask, np.float32).T)
        in_maps.append(m)
    return in_maps


def kernel(**inputs):
    from concourse.bass_utils import run_bass_kernel_spmd

    has_mask = bool(np.any(np.asarray(inputs["attn_mask"])))
    if has_mask not in _CACHE:
        _CACHE[has_mask] = _build(has_mask)
    nc = _CACHE[has_mask]

    in_maps = _prep_inputs(has_mask=has_mask, **inputs)
    res = run_bass_kernel_spmd(nc, in_maps, list(range(NCORES))).results
    full = np.concatenate([res[r]["out"] for r in range(NCORES)], axis=1)
    return full.reshape(B, S, H).astype(np.float32)


# revision 16
# speedup vs baseline: 1.3872x; 1.0476x over previous
"""MLA (DeepSeek-style multi-head latent attention) Bass kernel for 8 trn2 NeuronCores.

Sharding: tensor-parallel over heads (2 heads/core) for the big projections +
attention. The low-rank A-projections are CHANNEL-sharded (each core computes
256 of the 2048 hq+kv latent channels for all 2048 tokens) so the AllGathered
latents read back as contiguous 4KB runs; k_pe (64 rope channels) is computed
redundantly on every core, skipping it in the collective. RMS normalization
happens after the gather: per-core partial sum-of-squares rides the AllGather
as one extra bf16 row per channel block, and the rsqrt factors are folded into
the projection PSUM->SBUF copies (free-dim broadcast for q/k, per-partition
tensor_scalar for v). The output projection is column-parallel with a split
AllGather: head 0's attention output gathers and partially accumulates into
wo while head 1's attention still runs.

All tensors bf16 (fp32 PSUM accumulation); exp softmax without max-subtract.
Host-side (free) preprocessing: bf16 casts, weight transposes/permutations,
folding q_norm_w/kv_norm_w and SOFTMAX_SCALE into wq_b/wkv_b, rope sign folds.
"""

import math
import sys

import numpy as np

for _p in ("/opt/trn_rl_repo", "/root/.axon_site/_ro/trn_rl_repo"):
    if _p not in sys.path:
        sys.path.append(_p)

B, S, H = 1, 2048, 2048
NH = 16
Q_LORA, KV_LORA = 1536, 512
D_NOPE, D_ROPE, D_V = 128, 64, 128
D_QK = D_NOPE + D_ROPE
ROPE_FACTOR, MSCALE = 4.0, 1.0
SOFTMAX_SCALE = D_QK ** -0.5 * (0.1 * MSCALE * math.log(ROPE_FACTOR) + 1.0) ** 2
EPS = 1e-6

NCORES = 8
SSH = S // NCORES          # 256 output channels per core (wo column-parallel)
CTOT = Q_LORA + KV_LORA + D_ROPE   # 2112 latent channels

_CACHE = {}


def _build(has_mask: bool):
    import concourse.bacc as bacc
    import concourse.mybir as mybir
    import concourse.tile as tile

    f32 = mybir.dt.float32
    b16 = mybir.dt.bfloat16
    AF = mybir.ActivationFunctionType
    OP = mybir.AluOpType

    nc = bacc.Bacc("TRN2", target_bir_lowering=False, debug=False,
                   num_devices=NCORES)

    hidT = nc.dram_tensor("hidT", [16, 128, S], b16, kind="ExternalInput")
    a_own = nc.dram_tensor("a_own", [16, 128, 256], b16, kind="ExternalInput")
    a_pe = nc.dram_tensor("a_pe", [16, 128, 64], b16, kind="ExternalInput")
    cosT2 = nc.dram_tensor("cosT2", [128, S], b16, kind="ExternalInput")
    sinT2s = nc.dram_tensor("sinT2s", [128, S], b16, kind="ExternalInput")
    wqbT = nc.dram_tensor("wqbT", [12, 128, 384], b16, kind="ExternalInput")
    wkvbT = nc.dram_tensor("wkvbT", [4, 128, 512], b16, kind="ExternalInput")
    woT = nc.dram_tensor("woT", [16, 128, SSH], b16, kind="ExternalInput")
    ones_a = nc.dram_tensor("ones_a", [128, 1], b16, kind="ExternalInput")
    ones_bf = nc.dram_tensor("ones_bf", [1, 128], f32, kind="ExternalInput")
    selH_d = nc.dram_tensor("selH", [16, 1], b16, kind="ExternalInput")
    selC_d = nc.dram_tensor("selC", [16, 1], b16, kind="ExternalInput")
    if has_mask:
        maskT = nc.dram_tensor("maskT", [S, S], b16, kind="ExternalInput")
    out = nc.dram_tensor("out", [S, SSH], f32, kind="ExternalOutput")

    bounce1 = nc.dram_tensor("bounce1", [2, 129, S], b16)
    gath1 = nc.dram_tensor("gath1", [NCORES, 2, 129, S], b16,
                           addr_space="Shared")
    bounce2a = nc.dram_tensor("bounce2a", [128, S], b16)
    gath2a = nc.dram_tensor("gath2a", [NCORES, 128, S], b16,
                            addr_space="Shared")
    bounce2b = nc.dram_tensor("bounce2b", [128, S], b16)
    gath2b = nc.dram_tensor("gath2b", [NCORES, 128, S], b16,
                            addr_space="Shared")

    RG = [list(range(NCORES))]

    def mm(ps, lhsT, rhs, start, stop):
        nc.tensor.matmul(ps, lhsT, rhs, start=start, stop=stop)

    from contextlib import ExitStack
    with tile.TileContext(nc) as tc, ExitStack() as _st:
        constp = _st.enter_context(tc.tile_pool(name="const", bufs=1))
        ones_col = constp.tile([128, 1], b16)
        nc.sync.dma_start(ones_col[:], ones_a.ap())
        ones_row = constp.tile([1, 128], f32)
        nc.sync.dma_start(ones_row[:], ones_bf.ap())
        selh_sb = constp.tile([16, 1], b16)
        nc.sync.dma_start(selh_sb[:], selH_d.ap())
        selc_sb = constp.tile([16, 1], b16)
        nc.sync.dma_start(selc_sb[:], selC_d.ap())
        eps1 = constp.tile([1, 1], f32)
        nc.any.memset(eps1[:], EPS)
        eps_col = constp.tile([128, 1], f32)
        nc.any.memset(eps_col[:], EPS)

        # stage-1 weights prefetched up front (overlap with stage 0)
        s1wp = _st.enter_context(tc.tile_pool(name="s1w", bufs=1))
        wqb_sb = s1wp.tile([128, 12, 384], b16)
        nc.sync.dma_start(wqb_sb[:], wqbT.ap().rearrange("o p d -> p o d"))
        wkvb_sb = s1wp.tile([128, 4, 512], b16)
        nc.sync.dma_start(wkvb_sb[:], wkvbT.ap().rearrange("o p d -> p o d"))
        cos2_sb = s1wp.tile([128, S], b16)
        nc.sync.dma_start(cos2_sb[:], cosT2.ap())
        sin2_sb = s1wp.tile([128, S], b16)
        nc.sync.dma_start(sin2_sb[:], sinT2s.ap())
        wot_sb = s1wp.tile([128, 16, SSH], b16)   # DMA issued before attention

        # attention-lifetime pool (also covers the wo epilogue)
        attp = _st.enter_context(tc.tile_pool(name="att", bufs=1))
        kpe_sb = attp.tile([128, S], b16)
        nc.any.memset(kpe_sb[64:128, :], 0.0)

        # ---------------- stage 0: own 256 latent channels for all tokens
        with tc.tile_pool(name="s0", bufs=1) as s0p, \
             tc.tile_pool(name="s0ps", bufs=3, space="PSUM") as s0ps, \
             tc.tile_pool(name="s0ss", bufs=2, space="PSUM") as s0ssp, \
             tc.tile_pool(name="s0pe", bufs=2, space="PSUM") as s0pe, \
             tc.tile_pool(name="s0sq", bufs=3) as s0sqp:
            hid_sb = s0p.tile([128, 16, S], b16)
            for g in range(4):
                nc.sync.dma_start(
                    hid_sb[:, g * 4:(g + 1) * 4, :],
                    hidT.ap()[g * 4:(g + 1) * 4].rearrange("o p s -> p o s"))
            a_sb = s0p.tile([128, 16, 256], b16)
            nc.sync.dma_start(a_sb[:], a_own.ap().rearrange("o p c -> p o c"))
            ape_sb = s0p.tile([128, 16, 64], b16)
            nc.sync.dma_start(ape_sb[:], a_pe.ap().rearrange("o p c -> p o c"))

            raw = s0p.tile([128, 2, S], b16)
            kpraw = s0p.tile([64, S], b16)
            ssb16 = s0p.tile([1, S], b16)
            zrow = s0p.tile([1, S], b16)
            nc.any.memset(zrow[:], 0.0)
            for tt in range(4):
                sl = slice(tt * 512, (tt + 1) * 512)
                ssp = s0ssp.tile([1, 512], f32, tag="ss")
                for ct in range(2):
                    ps = s0ps.tile([128, 512], f32, tag="s0ps")
                    for hb in range(16):
                        mm(ps, a_sb[:, hb, ct * 128:(ct + 1) * 128],
                           hid_sb[:, hb, sl], hb == 0, hb == 15)
                    nc.vector.tensor_copy(raw[:, ct, sl], ps[:])
                    sq = s0sqp.tile([128, 512], b16, tag="s0sq")
                    nc.scalar.activation(sq[:], ps[:], AF.Square)
                    mm(ssp, ones_col, sq, ct == 0, ct == 1)
                nc.scalar.copy(ssb16[:, sl], ssp[:])
                kp = s0pe.tile([64, 512], f32, tag="kpe")
                for hb in range(16):
                    mm(kp, ape_sb[:, hb, :], hid_sb[:, hb, sl],
                       hb == 0, hb == 15)
                nc.vector.tensor_copy(kpraw[:, sl], kp[:])

            # k_pe rope (redundant on every core; not in the collective)
            t1 = s0p.tile([64, S], b16)
            nc.vector.tensor_tensor(t1[:], kpraw[:], cos2_sb[0:64, :], OP.mult)
            rsw = s0p.tile([64, S], b16)
            nc.sync.dma_start(rsw[0:32], kpraw[32:64])
            nc.sync.dma_start(rsw[32:64], kpraw[0:32])
            nc.vector.tensor_tensor(rsw[:], rsw[:], sin2_sb[0:64, :], OP.mult)
            nc.vector.tensor_tensor(kpe_sb[0:64, :], t1[:], rsw[:], OP.add)

            nc.sync.dma_start(
                bounce1.ap()[:, 0:128, :].rearrange("o p s -> p o s"), raw[:])
            nc.sync.dma_start(bounce1.ap()[0, 128:129, :], ssb16[:])
            nc.sync.dma_start(bounce1.ap()[1, 128:129, :], zrow[:])

        nc.gpsimd.collective_compute(
            "AllGather", OP.bypass, replica_groups=RG,
            ins=[bounce1.ap().opt()], outs=[gath1.ap().opt()])

        # ---------------- stage 1: per-head projections + attention
        qn0 = attp.tile([128, S], b16)
        qt1 = attp.tile([128, S], b16)
        qn1 = attp.tile([128, S], b16)
        qdst = (qn0, qt1, qn1)
        kn0 = attp.tile([128, S], b16)
        kn1 = attp.tile([128, S], b16)
        kn = (kn0, kn1)
        vt = [attp.tile([128, 256], b16, name=f"vt{tb}") for tb in range(16)]
        qt1r = attp.tile([128, S], b16)
        qr1 = attp.tile([128, S], b16)
        nc.any.memset(qr1[64:128, :], 0.0)
        xh0 = attp.tile([128, S], b16)
        xh1 = attp.tile([128, S], b16)

        with tc.tile_pool(name="proj", bufs=1) as prj, \
             tc.tile_pool(name="bcp", bufs=1) as bcp:
            # gathered reads: all contiguous 4KB runs
            partials = prj.tile([16, S], b16)
            nc.sync.dma_start(
                partials[:],
                gath1.ap()[:, :, 128, :].rearrange("a b s -> (a b) s"))
            hq_all = prj.tile([128, 12, S], b16)
            for r in range(6):
                nc.sync.dma_start(
                    hq_all[:, 2 * r:2 * r + 2, :],
                    gath1.ap()[r, :, 0:128, :].rearrange("o p s -> p o s"))
            kv_all = prj.tile([128, 4, S], b16)
            for r in (6, 7):
                nc.sync.dma_start(
                    kv_all[:, 2 * (r - 6):2 * (r - 6) + 2, :],
                    gath1.ap()[r, :, 0:128, :].rearrange("o p s -> p o s"))

            # rms factors: [1, S] free-layout (q/k) + [128, 16] col-layout (v)
            sq_hq = prj.tile([1, S], f32)
            sq_kv = prj.tile([1, S], f32)
            rc_hq_t = prj.tile([1, S], f32)
            rc_kv_t = prj.tile([1, S], f32)
            sqcol = prj.tile([128, 16], f32)
            rckv_col = prj.tile([128, 16], f32)
            bc_hq, bc_kv = [], []
            with tc.tile_pool(name="pfac", bufs=2, space="PSUM") as pfac, \
                 tc.tile_pool(name="pbc", bufs=2, space="PSUM") as pbc:
                for tt in range(4):
                    sl = slice(tt * 512, (tt + 1) * 512)
                    for selt, sqt in ((selh_sb, sq_hq), (selc_sb, sq_kv)):
                        ps2 = pfac.tile([1, 512], f32, tag="ps2")
                        mm(ps2, selt, partials[:, sl], True, True)
                        nc.scalar.activation(sqt[:, sl], ps2[:], AF.Sqrt,
                                             bias=eps1[:])
                nc.vector.reciprocal(rc_hq_t[:], sq_hq[:])
                nc.vector.reciprocal(rc_kv_t[:], sq_kv[:])
                pscol = pfac.tile([128, 16], f32, tag="pscol")
                for tb in range(16):
                    mm(pscol[:, tb:tb + 1],
                       partials[:, tb * 128:(tb + 1) * 128],
                       selc_sb, True, True)
                nc.scalar.activation(sqcol[:], pscol[:], AF.Sqrt,
                                     bias=eps_col[:])
                nc.vector.reciprocal(rckv_col[:], sqcol[:])

                # broadcast rsqrt factors to 128 partitions, per 512-tok chunk
                for tt in range(4):
                    sl = slice(tt * 512, (tt + 1) * 512)
                    for ty, rct, dst in ((0, rc_hq_t, bc_hq),
                                         (1, rc_kv_t, bc_kv)):
                        psb = pbc.tile([128, 512], f32, tag="pbc")
                        mm(psb, ones_row, rct[:, sl], True, True)
                        bt = bcp.tile([128, 512], f32, name=f"bc{ty}_{tt}")
                        nc.vector.tensor_copy(bt[:], psb[:])
                        dst.append(bt)

            # projections with normalization fused into the PSUM->SBUF step
            with tc.tile_pool(name="p1ps", bufs=3, space="PSUM") as p1ps, \
                 tc.tile_pool(name="p1psv", bufs=2, space="PSUM") as p1psv:
                for tt in range(4):
                    sl = slice(tt * 512, (tt + 1) * 512)
                    for m in range(3):
                        ps = p1ps.tile([128, 512], f32, tag="p1ps")
                        for cc in range(12):
                            mm(ps, wqb_sb[:, cc, m * 128:(m + 1) * 128],
                               hq_all[:, cc, sl], cc == 0, cc == 11)
                        nc.vector.tensor_tensor(qdst[m][:, sl], ps[:],
                                                bc_hq[tt][:], OP.mult)
                    for kh in range(2):
                        ps = p1ps.tile([128, 512], f32, tag="p1ps")
                        for cc in range(4):
                            mm(ps, wkvb_sb[:, cc, kh * 128:(kh + 1) * 128],
                               kv_all[:, cc, sl], cc == 0, cc == 3)
                        nc.vector.tensor_tensor(kn[kh][:, sl], ps[:],
                                                bc_kv[tt][:], OP.mult)
                for tb in range(16):
                    ps = p1psv.tile([128, 256], f32, tag="p1psv")
                    for cc in range(4):
                        mm(ps, kv_all[:, cc, tb * 128:(tb + 1) * 128],
                           wkvb_sb[:, cc, 256:512], cc == 0, cc == 3)
                        # lhsT = latent chunk [c,t], rhs = v cols of wkv_b'^T
                    nc.vector.tensor_scalar_mul(vt[tb][:], ps[:],
                                                rckv_col[:, tb:tb + 1])

                # rope on q (both heads share qt1: rows 0:64 h0, 64:128 h1)
                tmp = prj.tile([128, S], b16)
                for b in (0, 64):
                    nc.sync.dma_start(tmp[b:b + 32], qt1[b + 32:b + 64])
                    nc.sync.dma_start(tmp[b + 32:b + 64], qt1[b:b + 32])
                nc.vector.tensor_tensor(qt1r[:], qt1[:], cos2_sb[:], OP.mult)
                nc.vector.tensor_tensor(tmp[:], tmp[:], sin2_sb[:], OP.mult)
                nc.vector.tensor_tensor(qt1r[:], qt1r[:], tmp[:], OP.add)
                # h1 rope rows to base-0 tile (rows 64: zero; kpe rows 64: 0)
                nc.sync.dma_start(qr1[0:64, :], qt1r[64:128])

        # wo weights prefetch (overlaps attention)
        nc.sync.dma_start(wot_sb[:], woT.ap().rearrange("o p s -> p o s"))

        # attention + split AllGather + column-parallel wo
        with tc.tile_pool(name="apss", bufs=2, space="PSUM") as apss, \
             tc.tile_pool(name="apsx", bufs=2, space="PSUM") as apsx, \
             tc.tile_pool(name="apsd", bufs=1, space="PSUM") as apsd, \
             tc.tile_pool(name="apsb", bufs=1, space="PSUM") as apsb, \
             tc.tile_pool(name="wops", bufs=2, space="PSUM") as wops, \
             tc.tile_pool(name="aex", bufs=3) as aexp, \
             tc.tile_pool(name="asm", bufs=2) as asmp, \
             tc.tile_pool(name="amk", bufs=2) as amkp, \
             tc.tile_pool(name="wop", bufs=1) as wop, \
             tc.tile_pool(name="woot", bufs=3) as wootp:

            def attend(h, xh):
                qn_h = qn0 if h == 0 else qn1
                qr_h = qt1r if h == 0 else qr1
                for sb in range(4):
                    sl = slice(sb * 512, (sb + 1) * 512)
                    psx = apsx.tile([128, 512], f32, tag="apsx")
                    psd = apsd.tile([1, 512], f32, tag="apsd")
                    for tb in range(16):
                        tsl = slice(tb * 128, (tb + 1) * 128)
                        pss = apss.tile([128, 512], f32, tag="apss")
                        mm(pss, kn[h][:, tsl], qn_h[:, sl], True, False)
                        mm(pss, kpe_sb[:, tsl], qr_h[:, sl], False, True)
                        if has_mask:
                            mk = amkp.tile([128, 512], b16, tag="amk")
                            nc.sync.dma_start(
                                mk[:], maskT.ap()[tsl, sl])
                            nc.vector.tensor_tensor(pss[:], pss[:], mk[:],
                                                    OP.add)
                        ex = aexp.tile([128, 512], b16, tag="aex")
                        nc.scalar.activation(ex[:], pss[:], AF.Exp)
                        mm(psx, vt[tb][:, h * 128:(h + 1) * 128], ex,
                           tb == 0, tb == 15)
                        mm(psd, ones_col, ex, tb == 0, tb == 15)
                    rd = asmp.tile([1, 512], f32, tag="rd")
                    nc.vector.reciprocal(rd[:], psd[:])
                    psb2 = apsb.tile([128, 512], f32, tag="apsb")
                    mm(psb2, ones_row, rd, True, True)
                    rdb = asmp.tile([128, 512], f32, tag="rdb")
                    nc.vector.tensor_copy(rdb[:], psb2[:])
                    nc.vector.tensor_tensor(xh[:, sl], psx[:], rdb[:],
                                            OP.mult)

            attend(0, xh0)
            nc.sync.dma_start(bounce2a.ap(), xh0[:])
            nc.gpsimd.collective_compute(
                "AllGather", OP.bypass, replica_groups=RG,
                ins=[bounce2a.ap().opt()], outs=[gath2a.ap().opt()])

            # head-1 attention runs while gath2a lands + wo half-accumulates
            attend(1, xh1)

            xe_a = wop.tile([128, 8, S], b16)
            for r in range(NCORES):
                nc.sync.dma_start(xe_a[:, r, :], gath2a.ap()[r])
            opart = [wop.tile([128, SSH], f32, name=f"op{st}")
                     for st in range(16)]
            for st in range(16):
                pso = wops.tile([128, SSH], f32, tag="wops")
                for r in range(NCORES):
                    mm(pso, xe_a[:, r, st * 128:(st + 1) * 128],
                       wot_sb[:, 2 * r, :], r == 0, r == 7)
                nc.vector.tensor_copy(opart[st][:], pso[:])

            nc.sync.dma_start(bounce2b.ap(), xh1[:])
            nc.gpsimd.collective_compute(
                "AllGather", OP.bypass, replica_groups=RG,
                ins=[bounce2b.ap().opt()], outs=[gath2b.ap().opt()])

            xe_b = wop.tile([128, 8, S], b16)
            for r in range(NCORES):
                nc.sync.dma_start(xe_b[:, r, :], gath2b.ap()[r])
            for st in range(16):
                pso = wops.tile([128, SSH], f32, tag="wops")
                for r in range(NCORES):
                    mm(pso, xe_b[:, r, st * 128:(st + 1) * 128],
                       wot_sb[:, 2 * r + 1, :], r == 0, r == 7)
                ot = wootp.tile([128, SSH], f32, tag="ot")
                nc.vector.tensor_tensor(ot[:], pso[:], opart[st][:], OP.add)
                nc.sync.dma_start(out.ap()[st * 128:(st + 1) * 128, :], ot[:])

    nc.compile()
    return nc


def _prep_inputs(hidden_states, cos, sin, attn_mask, wq_a, q_norm_w, wq_b,
                 wkv_a, kv_norm_w, wkv_b, wo, has_mask):
    import ml_dtypes
    bf = ml_dtypes.bfloat16

    def c(x):
        return np.ascontiguousarray(x.astype(bf))

    hid = np.asarray(hidden_states, np.float32)[0]          # [S, H]
    hidT = c(hid.T.reshape(16, 128, S))                     # [H, S]
    A_T = np.vstack([np.asarray(wq_a, np.float32),
                     np.asarray(wkv_a, np.float32)]).T      # [H, CTOT]
    a_pe = c(A_T[:, 2048:2112].reshape(16, 128, 64))

    cosT = np.asarray(cos, np.float32).T                    # [64, S]
    sinT = np.asarray(sin, np.float32).T
    sinTs = sinT.copy()
    sinTs[0:32] *= -1.0
    cosT2 = c(np.concatenate([cosT, cosT], 0))              # [128, S]
    sinT2s = c(np.concatenate([sinTs, sinTs], 0))

    wqb = np.asarray(wq_b, np.float32) * np.asarray(q_norm_w, np.float32)[None]
    wqb = wqb * SOFTMAX_SCALE
    wkvb = (np.asarray(wkv_b, np.float32)
            * np.asarray(kv_norm_w, np.float32)[None])
    woT_full = np.asarray(wo, np.float32).T                 # [NH*DV, H]

    qperm = np.r_[0:128, 128:192, 320:384, 192:320]
    kvperm = np.r_[0:128, 256:384, 128:256, 384:512]

    selT = np.zeros((16, 2), np.float32)
    for p in range(0, 16, 2):
        if p < 12:
            selT[p, 0] = 1.0 / Q_LORA
        else:
            selT[p, 1] = 1.0 / KV_LORA
    selH = np.ascontiguousarray(selT[:, 0:1])
    selC = np.ascontiguousarray(selT[:, 1:2])

    in_maps = []
    for r in range(NCORES):
        m = {
            "hidT": hidT,
            "a_own": c(A_T[:, r * 256:(r + 1) * 256].reshape(16, 128, 256)),
            "a_pe": a_pe,
            "cosT2": cosT2,
            "sinT2s": sinT2s,
            "wqbT": c(wqb[r * 384:(r + 1) * 384].T[:, qperm]
                      .reshape(12, 128, 384)),
            "wkvbT": c(wkvb[r * 512:(r + 1) * 512].T[:, kvperm]
                       .reshape(4, 128, 512)),
            "woT": c(woT_full[:, r * SSH:(r + 1) * SSH].reshape(16, 128, SSH)),
            "ones_a": np.ones((128, 1), bf),
            "ones_bf": np.ones((1, 128), np.float32),
            "selH": c(selH),
            "selC": c(selC),
        }
        if has_mask:
            m["maskT"] = c(np.asarray(attn_mask, np.float32).T)
        in_maps.append(m)
    return in_maps


def kernel(**inputs):
    from concourse.bass_utils import run_bass_kernel_spmd

    has_mask = bool(np.any(np.asarray(inputs["attn_mask"])))
    if has_mask not in _CACHE:
        _CACHE[has_mask] = _build(has_mask)
    nc = _CACHE[has_mask]

    in_maps = _prep_inputs(has_mask=has_mask, **inputs)
    res = run_bass_kernel_spmd(nc, in_maps, list(range(NCORES))).results
    full = np.concatenate([res[r]["out"] for r in range(NCORES)], axis=1)
    return full.reshape(B, S, H).astype(np.float32)


# revision 17
# speedup vs baseline: 1.4220x; 1.0250x over previous
"""MLA (DeepSeek-style multi-head latent attention) Bass kernel for 8 trn2 NeuronCores.

Sharding: tensor-parallel over heads (2 heads/core) for the big projections +
attention. The low-rank A-projections are CHANNEL-sharded (each core computes
256 of the 2048 hq+kv latent channels for all 2048 tokens) so the AllGathered
latents read back as contiguous 4KB runs; k_pe (64 rope channels) is computed
redundantly on every core, skipping it in the collective. RMS normalization
happens after the gather: per-core partial sum-of-squares rides the AllGather
as one extra bf16 row per channel block, and the rsqrt factors are folded into
the projection PSUM->SBUF copies (free-dim broadcast for q/k, per-partition
tensor_scalar for v). The output projection is column-parallel with a split
AllGather: head 0's attention output gathers and partially accumulates into
wo while head 1's attention still runs.

All tensors bf16 (fp32 PSUM accumulation); exp softmax without max-subtract.
Host-side (free) preprocessing: bf16 casts, weight transposes/permutations,
folding q_norm_w/kv_norm_w and SOFTMAX_SCALE into wq_b/wkv_b, rope sign folds.
"""

import math
import sys

import numpy as np

for _p in ("/opt/trn_rl_repo", "/root/.axon_site/_ro/trn_rl_repo"):
    if _p not in sys.path:
        sys.path.append(_p)

B, S, H = 1, 2048, 2048
NH = 16
Q_LORA, KV_LORA = 1536, 512
D_NOPE, D_ROPE, D_V = 128, 64, 128
D_QK = D_NOPE + D_ROPE
ROPE_FACTOR, MSCALE = 4.0, 1.0
SOFTMAX_SCALE = D_QK ** -0.5 * (0.1 * MSCALE * math.log(ROPE_FACTOR) + 1.0) ** 2
EPS = 1e-6

NCORES = 8
SSH = S // NCORES          # 256 output channels per core (wo column-parallel)
CTOT = Q_LORA + KV_LORA + D_ROPE   # 2112 latent channels

_CACHE = {}


def _build(has_mask: bool):
    import concourse.bacc as bacc
    import concourse.mybir as mybir
    import concourse.tile as tile

    f32 = mybir.dt.float32
    b16 = mybir.dt.bfloat16
    AF = mybir.ActivationFunctionType
    OP = mybir.AluOpType

    nc = bacc.Bacc("TRN2", target_bir_lowering=False, debug=False,
                   num_devices=NCORES)

    hidT = nc.dram_tensor("hidT", [16, 128, S], b16, kind="ExternalInput")
    a_own = nc.dram_tensor("a_own", [16, 128, 256], b16, kind="ExternalInput")
    a_pe = nc.dram_tensor("a_pe", [16, 128, 64], b16, kind="ExternalInput")
    cosT2 = nc.dram_tensor("cosT2", [128, S], b16, kind="ExternalInput")
    sinT2s = nc.dram_tensor("sinT2s", [128, S], b16, kind="ExternalInput")
    wqbT = nc.dram_tensor("wqbT", [12, 128, 384], b16, kind="ExternalInput")
    wkvbT = nc.dram_tensor("wkvbT", [4, 128, 512], b16, kind="ExternalInput")
    woT = nc.dram_tensor("woT", [16, 128, SSH], b16, kind="ExternalInput")
    ones_a = nc.dram_tensor("ones_a", [128, 1], b16, kind="ExternalInput")
    ones_bf = nc.dram_tensor("ones_bf", [1, 128], f32, kind="ExternalInput")
    selH_d = nc.dram_tensor("selH", [16, 1], b16, kind="ExternalInput")
    selC_d = nc.dram_tensor("selC", [16, 1], b16, kind="ExternalInput")
    if has_mask:
        maskT = nc.dram_tensor("maskT", [S, S], b16, kind="ExternalInput")
    out = nc.dram_tensor("out", [S, SSH], f32, kind="ExternalOutput")

    bounce1 = nc.dram_tensor("bounce1", [2, 129, S], b16)
    gath1 = nc.dram_tensor("gath1", [NCORES, 2, 129, S], b16,
                           addr_space="Shared")
    bounce2a = nc.dram_tensor("bounce2a", [128, S], b16)
    gath2a = nc.dram_tensor("gath2a", [NCORES, 128, S], b16,
                            addr_space="Shared")
    bounce2b = nc.dram_tensor("bounce2b", [128, S], b16)
    gath2b = nc.dram_tensor("gath2b", [NCORES, 128, S], b16,
                            addr_space="Shared")

    RG = [list(range(NCORES))]

    def mm(ps, lhsT, rhs, start, stop):
        nc.tensor.matmul(ps, lhsT, rhs, start=start, stop=stop)

    from contextlib import ExitStack
    with tile.TileContext(nc) as tc, ExitStack() as _st:
        constp = _st.enter_context(tc.tile_pool(name="const", bufs=1))
        ones_col = constp.tile([128, 1], b16)
        nc.sync.dma_start(ones_col[:], ones_a.ap())
        ones_row = constp.tile([1, 128], f32)
        nc.sync.dma_start(ones_row[:], ones_bf.ap())
        selh_sb = constp.tile([16, 1], b16)
        nc.sync.dma_start(selh_sb[:], selH_d.ap())
        selc_sb = constp.tile([16, 1], b16)
        nc.sync.dma_start(selc_sb[:], selC_d.ap())
        eps1 = constp.tile([1, 1], f32)
        nc.any.memset(eps1[:], EPS)
        eps_col = constp.tile([128, 1], f32)
        nc.any.memset(eps_col[:], EPS)

        # stage-1 weight tiles (DMAs issued after stage-0's loads)
        s1wp = _st.enter_context(tc.tile_pool(name="s1w", bufs=1))
        wqb_sb = s1wp.tile([128, 12, 384], b16)
        wkvb_sb = s1wp.tile([128, 4, 512], b16)
        cos2_sb = s1wp.tile([128, S], b16)
        sin2_sb = s1wp.tile([128, S], b16)
        wot_sb = s1wp.tile([128, 16, SSH], b16)   # DMA issued before attention

        # attention-lifetime pool (also covers the wo epilogue)
        attp = _st.enter_context(tc.tile_pool(name="att", bufs=1))
        kpe_sb = attp.tile([128, S], b16)
        nc.any.memset(kpe_sb[64:128, :], 0.0)

        # ---------------- stage 0: own 256 latent channels for all tokens
        with tc.tile_pool(name="s0", bufs=1) as s0p, \
             tc.tile_pool(name="s0ps", bufs=3, space="PSUM") as s0ps, \
             tc.tile_pool(name="s0ss", bufs=2, space="PSUM") as s0ssp, \
             tc.tile_pool(name="s0pe", bufs=2, space="PSUM") as s0pe, \
             tc.tile_pool(name="s0sq", bufs=3) as s0sqp:
            hid_sb = s0p.tile([128, 16, S], b16)
            for g in range(4):
                nc.sync.dma_start(
                    hid_sb[:, g * 4:(g + 1) * 4, :],
                    hidT.ap()[g * 4:(g + 1) * 4].rearrange("o p s -> p o s"))
            a_sb = s0p.tile([128, 16, 256], b16)
            nc.sync.dma_start(a_sb[:], a_own.ap().rearrange("o p c -> p o c"))
            ape_sb = s0p.tile([128, 16, 64], b16)
            nc.sync.dma_start(ape_sb[:], a_pe.ap().rearrange("o p c -> p o c"))
            # stage-1 weights load behind stage-0's operands
            nc.sync.dma_start(cos2_sb[:], cosT2.ap())
            nc.sync.dma_start(sin2_sb[:], sinT2s.ap())
            nc.sync.dma_start(wqb_sb[:],
                              wqbT.ap().rearrange("o p d -> p o d"))
            nc.sync.dma_start(wkvb_sb[:],
                              wkvbT.ap().rearrange("o p d -> p o d"))

            raw = s0p.tile([128, 2, S], b16)
            kpraw = s0p.tile([64, S], b16)
            ssb16 = s0p.tile([1, S], b16)
            zrow = s0p.tile([1, S], b16)
            nc.any.memset(zrow[:], 0.0)
            for tt in range(4):
                sl = slice(tt * 512, (tt + 1) * 512)
                ssp = s0ssp.tile([1, 512], f32, tag="ss")
                for ct in range(2):
                    ps = s0ps.tile([128, 512], f32, tag="s0ps")
                    for hb in range(16):
                        mm(ps, a_sb[:, hb, ct * 128:(ct + 1) * 128],
                           hid_sb[:, hb, sl], hb == 0, hb == 15)
                    nc.vector.tensor_copy(raw[:, ct, sl], ps[:])
                    sq = s0sqp.tile([128, 512], b16, tag="s0sq")
                    nc.scalar.activation(sq[:], ps[:], AF.Square)
                    mm(ssp, ones_col, sq, ct == 0, ct == 1)
                nc.scalar.copy(ssb16[:, sl], ssp[:])
                kp = s0pe.tile([64, 512], f32, tag="kpe")
                for hb in range(16):
                    mm(kp, ape_sb[:, hb, :], hid_sb[:, hb, sl],
                       hb == 0, hb == 15)
                nc.vector.tensor_copy(kpraw[:, sl], kp[:])

            # k_pe rope (redundant on every core; not in the collective)
            t1 = s0p.tile([64, S], b16)
            nc.vector.tensor_tensor(t1[:], kpraw[:], cos2_sb[0:64, :], OP.mult)
            rsw = s0p.tile([64, S], b16)
            nc.sync.dma_start(rsw[0:32], kpraw[32:64])
            nc.sync.dma_start(rsw[32:64], kpraw[0:32])
            nc.vector.tensor_tensor(rsw[:], rsw[:], sin2_sb[0:64, :], OP.mult)
            nc.vector.tensor_tensor(kpe_sb[0:64, :], t1[:], rsw[:], OP.add)

            nc.sync.dma_start(
                bounce1.ap()[:, 0:128, :].rearrange("o p s -> p o s"), raw[:])
            nc.sync.dma_start(bounce1.ap()[0, 128:129, :], ssb16[:])
            nc.sync.dma_start(bounce1.ap()[1, 128:129, :], zrow[:])

        nc.gpsimd.collective_compute(
            "AllGather", OP.bypass, replica_groups=RG,
            ins=[bounce1.ap().opt()], outs=[gath1.ap().opt()])

        # ---------------- stage 1: per-head projections + attention
        qn0 = attp.tile([128, S], b16)
        qt1 = attp.tile([128, S], b16)
        qn1 = attp.tile([128, S], b16)
        qdst = (qn0, qt1, qn1)
        kn0 = attp.tile([128, S], b16)
        kn1 = attp.tile([128, S], b16)
        kn = (kn0, kn1)
        vt = [attp.tile([128, 256], b16, name=f"vt{tb}") for tb in range(16)]
        qt1r = attp.tile([128, S], b16)
        qr1 = attp.tile([128, S], b16)
        nc.any.memset(qr1[64:128, :], 0.0)
        xh0 = attp.tile([128, S], b16)
        xh1 = attp.tile([128, S], b16)

        with tc.tile_pool(name="proj", bufs=1) as prj, \
             tc.tile_pool(name="bcp", bufs=1) as bcp:
            # gathered reads: all contiguous 4KB runs
            partials = prj.tile([16, S], b16)
            nc.sync.dma_start(
                partials[:],
                gath1.ap()[:, :, 128, :].rearrange("a b s -> (a b) s"))
            hq_all = prj.tile([128, 12, S], b16)
            for r in range(6):
                nc.sync.dma_start(
                    hq_all[:, 2 * r:2 * r + 2, :],
                    gath1.ap()[r, :, 0:128, :].rearrange("o p s -> p o s"))
            kv_all = prj.tile([128, 4, S], b16)
            for r in (6, 7):
                nc.sync.dma_start(
                    kv_all[:, 2 * (r - 6):2 * (r - 6) + 2, :],
                    gath1.ap()[r, :, 0:128, :].rearrange("o p s -> p o s"))

            # rms factors: [1, S] free-layout (q/k) + [128, 16] col-layout (v)
            sq_hq = prj.tile([1, S], f32)
            sq_kv = prj.tile([1, S], f32)
            rc_hq_t = prj.tile([1, S], f32)
            rc_kv_t = prj.tile([1, S], f32)
            sqcol = prj.tile([128, 16], f32)
            rckv_col = prj.tile([128, 16], f32)
            bc_hq, bc_kv = [], []
            with tc.tile_pool(name="pfac", bufs=2, space="PSUM") as pfac, \
                 tc.tile_pool(name="pbc", bufs=2, space="PSUM") as pbc:
                for tt in range(4):
                    sl = slice(tt * 512, (tt + 1) * 512)
                    for selt, sqt in ((selh_sb, sq_hq), (selc_sb, sq_kv)):
                        ps2 = pfac.tile([1, 512], f32, tag="ps2")
                        mm(ps2, selt, partials[:, sl], True, True)
                        nc.scalar.activation(sqt[:, sl], ps2[:], AF.Sqrt,
                                             bias=eps1[:])
                nc.vector.reciprocal(rc_hq_t[:], sq_hq[:])
                nc.vector.reciprocal(rc_kv_t[:], sq_kv[:])
                pscol = pfac.tile([128, 16], f32, tag="pscol")
                for tb in range(16):
                    mm(pscol[:, tb:tb + 1],
                       partials[:, tb * 128:(tb + 1) * 128],
                       selc_sb, True, True)
                nc.scalar.activation(sqcol[:], pscol[:], AF.Sqrt,
                                     bias=eps_col[:])
                nc.vector.reciprocal(rckv_col[:], sqcol[:])

                # broadcast rsqrt factors to 128 partitions, per 512-tok chunk
                for tt in range(4):
                    sl = slice(tt * 512, (tt + 1) * 512)
                    for ty, rct, dst in ((0, rc_hq_t, bc_hq),
                                         (1, rc_kv_t, bc_kv)):
                        psb = pbc.tile([128, 512], f32, tag="pbc")
                        mm(psb, ones_row, rct[:, sl], True, True)
                        bt = bcp.tile([128, 512], f32, name=f"bc{ty}_{tt}")
                        nc.vector.tensor_copy(bt[:], psb[:])
                        dst.append(bt)

            # projections with normalization fused into the PSUM->SBUF step
            with tc.tile_pool(name="p1ps", bufs=3, space="PSUM") as p1ps, \
                 tc.tile_pool(name="p1psv", bufs=2, space="PSUM") as p1psv:
                for tt in range(4):
                    sl = slice(tt * 512, (tt + 1) * 512)
                    for m in range(3):
                        ps = p1ps.tile([128, 512], f32, tag="p1ps")
                        for cc in range(12):
                            mm(ps, wqb_sb[:, cc, m * 128:(m + 1) * 128],
                               hq_all[:, cc, sl], cc == 0, cc == 11)
                        nc.vector.tensor_tensor(qdst[m][:, sl], ps[:],
                                                bc_hq[tt][:], OP.mult)
                    for kh in range(2):
                        ps = p1ps.tile([128, 512], f32, tag="p1ps")
                        for cc in range(4):
                            mm(ps, wkvb_sb[:, cc, kh * 128:(kh + 1) * 128],
                               kv_all[:, cc, sl], cc == 0, cc == 3)
                        nc.vector.tensor_tensor(kn[kh][:, sl], ps[:],
                                                bc_kv[tt][:], OP.mult)
                for tb in range(16):
                    ps = p1psv.tile([128, 256], f32, tag="p1psv")
                    for cc in range(4):
                        mm(ps, kv_all[:, cc, tb * 128:(tb + 1) * 128],
                           wkvb_sb[:, cc, 256:512], cc == 0, cc == 3)
                        # lhsT = latent chunk [c,t], rhs = v cols of wkv_b'^T
                    nc.vector.tensor_scalar_mul(vt[tb][:], ps[:],
                                                rckv_col[:, tb:tb + 1])

                # rope on q (both heads share qt1: rows 0:64 h0, 64:128 h1)
                tmp = prj.tile([128, S], b16)
                for b in (0, 64):
                    nc.sync.dma_start(tmp[b:b + 32], qt1[b + 32:b + 64])
                    nc.sync.dma_start(tmp[b + 32:b + 64], qt1[b:b + 32])
                nc.vector.tensor_tensor(qt1r[:], qt1[:], cos2_sb[:], OP.mult)
                nc.vector.tensor_tensor(tmp[:], tmp[:], sin2_sb[:], OP.mult)
                nc.vector.tensor_tensor(qt1r[:], qt1r[:], tmp[:], OP.add)
                # h1 rope rows to base-0 tile (rows 64: zero; kpe rows 64: 0)
                nc.sync.dma_start(qr1[0:64, :], qt1r[64:128])

        # wo weights prefetch (overlaps attention)
        nc.sync.dma_start(wot_sb[:], woT.ap().rearrange("o p s -> p o s"))

        # attention + split AllGather + column-parallel wo
        with tc.tile_pool(name="apss", bufs=3, space="PSUM") as apss, \
             tc.tile_pool(name="apsx", bufs=2, space="PSUM") as apsx, \
             tc.tile_pool(name="apsd", bufs=1, space="PSUM") as apsd, \
             tc.tile_pool(name="apsb", bufs=1, space="PSUM") as apsb, \
             tc.tile_pool(name="wops", bufs=1, space="PSUM") as wops, \
             tc.tile_pool(name="aex", bufs=3) as aexp, \
             tc.tile_pool(name="asm", bufs=2) as asmp, \
             tc.tile_pool(name="amk", bufs=2) as amkp, \
             tc.tile_pool(name="wop", bufs=1) as wop, \
             tc.tile_pool(name="woot", bufs=3) as wootp:

            LOOKAHEAD = 2

            def attend(h, xh):
                qn_h = qn0 if h == 0 else qn1
                qr_h = qt1r if h == 0 else qr1
                for sb in range(4):
                    sl = slice(sb * 512, (sb + 1) * 512)
                    psx = apsx.tile([128, 512], f32, tag="apsx")
                    psd = apsd.tile([1, 512], f32, tag="apsd")
                    exq = []

                    def consume(tb, ex):
                        mm(psx, vt[tb][:, h * 128:(h + 1) * 128], ex,
                           tb == 0, tb == 15)
                        mm(psd, ones_col, ex, tb == 0, tb == 15)

                    for tb in range(16):
                        tsl = slice(tb * 128, (tb + 1) * 128)
                        pss = apss.tile([128, 512], f32, tag="apss")
                        mm(pss, kn[h][:, tsl], qn_h[:, sl], True, False)
                        mm(pss, kpe_sb[:, tsl], qr_h[:, sl], False, True)
                        if has_mask:
                            mk = amkp.tile([128, 512], b16, tag="amk")
                            nc.sync.dma_start(
                                mk[:], maskT.ap()[tsl, sl])
                            nc.vector.tensor_tensor(pss[:], pss[:], mk[:],
                                                    OP.add)
                        ex = aexp.tile([128, 512], b16, tag="aex")
                        nc.scalar.activation(ex[:], pss[:], AF.Exp)
                        exq.append((tb, ex))
                        if len(exq) > LOOKAHEAD:
                            consume(*exq.pop(0))
                    for item in exq:
                        consume(*item)
                    rd = asmp.tile([1, 512], f32, tag="rd")
                    nc.vector.reciprocal(rd[:], psd[:])
                    psb2 = apsb.tile([128, 512], f32, tag="apsb")
                    mm(psb2, ones_row, rd, True, True)
                    rdb = asmp.tile([128, 512], f32, tag="rdb")
                    nc.vector.tensor_copy(rdb[:], psb2[:])
                    nc.vector.tensor_tensor(xh[:, sl], psx[:], rdb[:],
                                            OP.mult)

            attend(0, xh0)
            nc.sync.dma_start(bounce2a.ap(), xh0[:])
            nc.gpsimd.collective_compute(
                "AllGather", OP.bypass, replica_groups=RG,
                ins=[bounce2a.ap().opt()], outs=[gath2a.ap().opt()])

            # head-1 attention runs while gath2a lands + wo half-accumulates
            attend(1, xh1)

            xe_a = wop.tile([128, 8, S], b16)
            for r in range(NCORES):
                nc.sync.dma_start(xe_a[:, r, :], gath2a.ap()[r])
            opart = [wop.tile([128, SSH], f32, name=f"op{st}")
                     for st in range(16)]
            for st in range(16):
                pso = wops.tile([128, SSH], f32, tag="wops")
                for r in range(NCORES):
                    mm(pso, xe_a[:, r, st * 128:(st + 1) * 128],
                       wot_sb[:, 2 * r, :], r == 0, r == 7)
                nc.vector.tensor_copy(opart[st][:], pso[:])

            nc.sync.dma_start(bounce2b.ap(), xh1[:])
            nc.gpsimd.collective_compute(
                "AllGather", OP.bypass, replica_groups=RG,
                ins=[bounce2b.ap().opt()], outs=[gath2b.ap().opt()])

            xe_b = wop.tile([128, 8, S], b16)
            for r in range(NCORES):
                nc.sync.dma_start(xe_b[:, r, :], gath2b.ap()[r])
            for st in range(16):
                pso = wops.tile([128, SSH], f32, tag="wops")
                for r in range(NCORES):
                    mm(pso, xe_b[:, r, st * 128:(st + 1) * 128],
                       wot_sb[:, 2 * r + 1, :], r == 0, r == 7)
                ot = wootp.tile([128, SSH], f32, tag="ot")
                nc.vector.tensor_tensor(ot[:], pso[:], opart[st][:], OP.add)
                nc.sync.dma_start(out.ap()[st * 128:(st + 1) * 128, :], ot[:])

    nc.compile()
    return nc


def _prep_inputs(hidden_states, cos, sin, attn_mask, wq_a, q_norm_w, wq_b,
                 wkv_a, kv_norm_w, wkv_b, wo, has_mask):
    import ml_dtypes
    bf = ml_dtypes.bfloat16

    def c(x):
        return np.ascontiguousarray(x.astype(bf))

    hid = np.asarray(hidden_states, np.float32)[0]          # [S, H]
    hidT = c(hid.T.reshape(16, 128, S))                     # [H, S]
    A_T = np.vstack([np.asarray(wq_a, np.float32),
                     np.asarray(wkv_a, np.float32)]).T      # [H, CTOT]
    a_pe = c(A_T[:, 2048:2112].reshape(16, 128, 64))

    cosT = np.asarray(cos, np.float32).T                    # [64, S]
    sinT = np.asarray(sin, np.float32).T
    sinTs = sinT.copy()
    sinTs[0:32] *= -1.0
    cosT2 = c(np.concatenate([cosT, cosT], 0))              # [128, S]
    sinT2s = c(np.concatenate([sinTs, sinTs], 0))

    wqb = np.asarray(wq_b, np.float32) * np.asarray(q_norm_w, np.float32)[None]
    wqb = wqb * SOFTMAX_SCALE
    wkvb = (np.asarray(wkv_b, np.float32)
            * np.asarray(kv_norm_w, np.float32)[None])
    woT_full = np.asarray(wo, np.float32).T                 # [NH*DV, H]

    qperm = np.r_[0:128, 128:192, 320:384, 192:320]
    kvperm = np.r_[0:128, 256:384, 128:256, 384:512]

    selT = np.zeros((16, 2), np.float32)
    for p in range(0, 16, 2):
        if p < 12:
            selT[p, 0] = 1.0 / Q_LORA
        else:
            selT[p, 1] = 1.0 / KV_LORA
    selH = np.ascontiguousarray(selT[:, 0:1])
    selC = np.ascontiguousarray(selT[:, 1:2])

    in_maps = []
    for r in range(NCORES):
        m = {
            "hidT": hidT,
            "a_own": c(A_T[:, r * 256:(r + 1) * 256].reshape(16, 128, 256)),
            "a_pe": a_pe,
            "cosT2": cosT2,
            "sinT2s": sinT2s,
            "wqbT": c(wqb[r * 384:(r + 1) * 384].T[:, qperm]
                      .reshape(12, 128, 384)),
            "wkvbT": c(wkvb[r * 512:(r + 1) * 512].T[:, kvperm]
                       .reshape(4, 128, 512)),
            "woT": c(woT_full[:, r * SSH:(r + 1) * SSH].reshape(16, 128, SSH)),
            "ones_a": np.ones((128, 1), bf),
            "ones_bf": np.ones((1, 128), np.float32),
            "selH": c(selH),
            "selC": c(selC),
        }
        if has_mask:
            m["maskT"] = c(np.asarray(attn_mask, np.float32).T)
        in_maps.append(m)
    return in_maps


def kernel(**inputs):
    from concourse.bass_utils import run_bass_kernel_spmd

    has_mask = bool(np.any(np.asarray(inputs["attn_mask"])))
    if has_mask not in _CACHE:
        _CACHE[has_mask] = _build(has_mask)
    nc = _CACHE[has_mask]

    in_maps = _prep_inputs(has_mask=has_mask, **inputs)
    res = run_bass_kernel_spmd(nc, in_maps, list(range(NCORES))).results
    full = np.concatenate([res[r]["out"] for r in range(NCORES)], axis=1)
    return full.reshape(B, S, H).astype(np.float32)


# revision 19
# speedup vs baseline: 1.4344x; 1.0088x over previous
"""MLA (DeepSeek-style multi-head latent attention) Bass kernel for 8 trn2 NeuronCores.

Sharding: tensor-parallel over heads (2 heads/core) for the big projections +
attention. The low-rank A-projections are CHANNEL-sharded (each core computes
256 of the 2048 hq+kv latent channels for all 2048 tokens) so the AllGathered
latents read back as contiguous 4KB runs; k_pe (64 rope channels) is computed
redundantly on every core, skipping it in the collective. RMS normalization
happens after the gather: per-core partial sum-of-squares rides the AllGather
as one extra bf16 row per channel block, and the rsqrt factors are folded into
the projection PSUM->SBUF copies (free-dim broadcast for q/k, per-partition
tensor_scalar for v). The output projection is column-parallel with a split
AllGather: head 0's attention output gathers and partially accumulates into
wo while head 1's attention still runs.

All tensors bf16 (fp32 PSUM accumulation); exp softmax without max-subtract.
Host-side (free) preprocessing: bf16 casts, weight transposes/permutations,
folding q_norm_w/kv_norm_w and SOFTMAX_SCALE into wq_b/wkv_b, rope sign folds.
"""

import math
import sys

import numpy as np

for _p in ("/opt/trn_rl_repo", "/root/.axon_site/_ro/trn_rl_repo"):
    if _p not in sys.path:
        sys.path.append(_p)

B, S, H = 1, 2048, 2048
NH = 16
Q_LORA, KV_LORA = 1536, 512
D_NOPE, D_ROPE, D_V = 128, 64, 128
D_QK = D_NOPE + D_ROPE
ROPE_FACTOR, MSCALE = 4.0, 1.0
SOFTMAX_SCALE = D_QK ** -0.5 * (0.1 * MSCALE * math.log(ROPE_FACTOR) + 1.0) ** 2
EPS = 1e-6

NCORES = 8
SSH = S // NCORES          # 256 output channels per core (wo column-parallel)
CTOT = Q_LORA + KV_LORA + D_ROPE   # 2112 latent channels

_CACHE = {}


def _build(has_mask: bool):
    import concourse.bacc as bacc
    import concourse.mybir as mybir
    import concourse.tile as tile

    f32 = mybir.dt.float32
    b16 = mybir.dt.bfloat16
    AF = mybir.ActivationFunctionType
    OP = mybir.AluOpType

    nc = bacc.Bacc("TRN2", target_bir_lowering=False, debug=False,
                   num_devices=NCORES)

    hidT = nc.dram_tensor("hidT", [16, 128, S], b16, kind="ExternalInput")
    a_own = nc.dram_tensor("a_own", [16, 128, 256], b16, kind="ExternalInput")
    a_pe = nc.dram_tensor("a_pe", [16, 128, 64], b16, kind="ExternalInput")
    cosT2 = nc.dram_tensor("cosT2", [128, S], b16, kind="ExternalInput")
    sinT2s = nc.dram_tensor("sinT2s", [128, S], b16, kind="ExternalInput")
    wqbT = nc.dram_tensor("wqbT", [12, 128, 384], b16, kind="ExternalInput")
    wkvbT = nc.dram_tensor("wkvbT", [4, 128, 512], b16, kind="ExternalInput")
    woT = nc.dram_tensor("woT", [16, 128, SSH], b16, kind="ExternalInput")
    ones_a = nc.dram_tensor("ones_a", [128, 1], b16, kind="ExternalInput")
    ones_bf = nc.dram_tensor("ones_bf", [1, 128], f32, kind="ExternalInput")
    selH_d = nc.dram_tensor("selH", [16, 1], b16, kind="ExternalInput")
    selC_d = nc.dram_tensor("selC", [16, 1], b16, kind="ExternalInput")
    if has_mask:
        maskT = nc.dram_tensor("maskT", [S, S], b16, kind="ExternalInput")
    out = nc.dram_tensor("out", [S, SSH], f32, kind="ExternalOutput")

    bounce1 = nc.dram_tensor("bounce1", [2, 129, S], b16)
    gath1 = nc.dram_tensor("gath1", [NCORES, 2, 129, S], b16,
                           addr_space="Shared")
    bounce2a = nc.dram_tensor("bounce2a", [128, S], b16)
    gath2a = nc.dram_tensor("gath2a", [NCORES, 128, S], b16,
                            addr_space="Shared")
    bounce2b = nc.dram_tensor("bounce2b", [128, S], b16)
    gath2b = nc.dram_tensor("gath2b", [NCORES, 128, S], b16,
                            addr_space="Shared")

    RG = [list(range(NCORES))]

    def mm(ps, lhsT, rhs, start, stop):
        nc.tensor.matmul(ps, lhsT, rhs, start=start, stop=stop)

    from contextlib import ExitStack
    with tile.TileContext(nc) as tc, ExitStack() as _st:
        constp = _st.enter_context(tc.tile_pool(name="const", bufs=1))
        ones_col = constp.tile([128, 1], b16)
        nc.sync.dma_start(ones_col[:], ones_a.ap())
        ones_row = constp.tile([1, 128], f32)
        nc.sync.dma_start(ones_row[:], ones_bf.ap())
        selh_sb = constp.tile([16, 1], b16)
        nc.sync.dma_start(selh_sb[:], selH_d.ap())
        selc_sb = constp.tile([16, 1], b16)
        nc.sync.dma_start(selc_sb[:], selC_d.ap())
        eps1 = constp.tile([1, 1], f32)
        nc.any.memset(eps1[:], EPS)
        eps_col = constp.tile([128, 1], f32)
        nc.any.memset(eps_col[:], EPS)

        # stage-1 weight tiles (DMAs issued after stage-0's loads)
        s1wp = _st.enter_context(tc.tile_pool(name="s1w", bufs=1))
        wqb_sb = s1wp.tile([128, 12, 384], b16)
        wkvb_sb = s1wp.tile([128, 4, 512], b16)
        cos2_sb = s1wp.tile([128, S], b16)
        sin2_sb = s1wp.tile([128, S], b16)
        wot_sb = s1wp.tile([128, 16, SSH], b16)   # DMA issued before attention

        # attention-lifetime pool (also covers the wo epilogue)
        attp = _st.enter_context(tc.tile_pool(name="att", bufs=1))
        kpe_sb = attp.tile([128, S], b16)
        nc.any.memset(kpe_sb[64:128, :], 0.0)

        # ---------------- stage 0: own 256 latent channels for all tokens
        with tc.tile_pool(name="s0", bufs=1) as s0p, \
             tc.tile_pool(name="s0ps", bufs=3, space="PSUM") as s0ps, \
             tc.tile_pool(name="s0ss", bufs=2, space="PSUM") as s0ssp, \
             tc.tile_pool(name="s0pe", bufs=2, space="PSUM") as s0pe, \
             tc.tile_pool(name="s0sq", bufs=3) as s0sqp:
            hid_sb = s0p.tile([128, 16, S], b16)
            for g in range(4):
                nc.sync.dma_start(
                    hid_sb[:, g * 4:(g + 1) * 4, :],
                    hidT.ap()[g * 4:(g + 1) * 4].rearrange("o p s -> p o s"))
            a_sb = s0p.tile([128, 16, 256], b16)
            nc.sync.dma_start(a_sb[:], a_own.ap().rearrange("o p c -> p o c"))
            ape_sb = s0p.tile([128, 16, 64], b16)
            nc.sync.dma_start(ape_sb[:], a_pe.ap().rearrange("o p c -> p o c"))
            # stage-1 weights load behind stage-0's operands
            nc.sync.dma_start(cos2_sb[:], cosT2.ap())
            nc.sync.dma_start(sin2_sb[:], sinT2s.ap())
            nc.sync.dma_start(wqb_sb[:],
                              wqbT.ap().rearrange("o p d -> p o d"))
            nc.sync.dma_start(wkvb_sb[:],
                              wkvbT.ap().rearrange("o p d -> p o d"))

            raw = s0p.tile([128, 2, S], b16)
            kpraw = s0p.tile([64, S], b16)
            ssb16 = s0p.tile([1, S], b16)
            zrow = s0p.tile([1, S], b16)
            nc.any.memset(zrow[:], 0.0)
            for tt in range(4):
                sl = slice(tt * 512, (tt + 1) * 512)
                ssp = s0ssp.tile([1, 512], f32, tag="ss")
                for ct in range(2):
                    ps = s0ps.tile([128, 512], f32, tag="s0ps")
                    for hb in range(16):
                        mm(ps, a_sb[:, hb, ct * 128:(ct + 1) * 128],
                           hid_sb[:, hb, sl], hb == 0, hb == 15)
                    nc.vector.tensor_copy(raw[:, ct, sl], ps[:])
                    sq = s0sqp.tile([128, 512], b16, tag="s0sq")
                    nc.scalar.activation(sq[:], ps[:], AF.Square)
                    mm(ssp, ones_col, sq, ct == 0, ct == 1)
                nc.scalar.copy(ssb16[:, sl], ssp[:])
                kp = s0pe.tile([64, 512], f32, tag="kpe")
                for hb in range(16):
                    mm(kp, ape_sb[:, hb, :], hid_sb[:, hb, sl],
                       hb == 0, hb == 15)
                nc.vector.tensor_copy(kpraw[:, sl], kp[:])

            # k_pe rope (redundant on every core; not in the collective)
            t1 = s0p.tile([64, S], b16)
            nc.vector.tensor_tensor(t1[:], kpraw[:], cos2_sb[0:64, :], OP.mult)
            rsw = s0p.tile([64, S], b16)
            nc.sync.dma_start(rsw[0:32], kpraw[32:64])
            nc.sync.dma_start(rsw[32:64], kpraw[0:32])
            nc.vector.tensor_tensor(rsw[:], rsw[:], sin2_sb[0:64, :], OP.mult)
            nc.vector.tensor_tensor(kpe_sb[0:64, :], t1[:], rsw[:], OP.add)

            nc.sync.dma_start(
                bounce1.ap()[:, 0:128, :].rearrange("o p s -> p o s"), raw[:])
            nc.sync.dma_start(bounce1.ap()[0, 128:129, :], ssb16[:])
            nc.sync.dma_start(bounce1.ap()[1, 128:129, :], zrow[:])

        nc.gpsimd.collective_compute(
            "AllGather", OP.bypass, replica_groups=RG,
            ins=[bounce1.ap().opt()], outs=[gath1.ap().opt()])

        # ---------------- stage 1: per-head projections + attention
        qn0 = attp.tile([128, S], b16)
        qt1 = attp.tile([128, S], b16)
        qn1 = attp.tile([128, S], b16)
        qdst = (qn0, qt1, qn1)
        kn0 = attp.tile([128, S], b16)
        kn1 = attp.tile([128, S], b16)
        kn = (kn0, kn1)
        vt = [attp.tile([128, 256], b16, name=f"vt{tb}") for tb in range(16)]
        qt1r = attp.tile([128, S], b16)
        qr1 = attp.tile([128, S], b16)
        nc.any.memset(qr1[64:128, :], 0.0)
        xh0 = attp.tile([128, S], b16)
        xh1 = attp.tile([128, S], b16)

        with tc.tile_pool(name="proj", bufs=1) as prj, \
             tc.tile_pool(name="bcp", bufs=1) as bcp:
            # gathered reads: all contiguous 4KB runs
            partials = prj.tile([16, S], b16)
            nc.sync.dma_start(
                partials[:],
                gath1.ap()[:, :, 128, :].rearrange("a b s -> (a b) s"))
            hq_all = prj.tile([128, 12, S], b16)
            for r in range(6):
                nc.sync.dma_start(
                    hq_all[:, 2 * r:2 * r + 2, :],
                    gath1.ap()[r, :, 0:128, :].rearrange("o p s -> p o s"))
            kv_all = prj.tile([128, 4, S], b16)
            for r in (6, 7):
                nc.sync.dma_start(
                    kv_all[:, 2 * (r - 6):2 * (r - 6) + 2, :],
                    gath1.ap()[r, :, 0:128, :].rearrange("o p s -> p o s"))

            # rms factors: [1, S] free-layout (q/k) + [128, 16] col-layout (v)
            sq_hq = prj.tile([1, S], f32)
            sq_kv = prj.tile([1, S], f32)
            rc_hq_t = prj.tile([1, S], f32)
            rc_kv_t = prj.tile([1, S], f32)
            sqcol = prj.tile([128, 16], f32)
            rckv_col = prj.tile([128, 16], f32)
            bc_hq, bc_kv = [], []
            with tc.tile_pool(name="pfac", bufs=2, space="PSUM") as pfac, \
                 tc.tile_pool(name="pbc", bufs=2, space="PSUM") as pbc:
                for tt in range(4):
                    sl = slice(tt * 512, (tt + 1) * 512)
                    for selt, sqt in ((selh_sb, sq_hq), (selc_sb, sq_kv)):
                        ps2 = pfac.tile([1, 512], f32, tag="ps2")
                        mm(ps2, selt, partials[:, sl], True, True)
                        nc.scalar.activation(sqt[:, sl], ps2[:], AF.Sqrt,
                                             bias=eps1[:])
                nc.vector.reciprocal(rc_hq_t[:], sq_hq[:])
                nc.vector.reciprocal(rc_kv_t[:], sq_kv[:])
                pscol = pfac.tile([128, 16], f32, tag="pscol")
                for tb in range(16):
                    mm(pscol[:, tb:tb + 1],
                       partials[:, tb * 128:(tb + 1) * 128],
                       selc_sb, True, True)
                nc.scalar.activation(sqcol[:], pscol[:], AF.Sqrt,
                                     bias=eps_col[:])
                nc.vector.reciprocal(rckv_col[:], sqcol[:])

                # broadcast rsqrt factors to 128 partitions, per 512-tok chunk
                for tt in range(4):
                    sl = slice(tt * 512, (tt + 1) * 512)
                    for ty, rct, dst in ((0, rc_hq_t, bc_hq),
                                         (1, rc_kv_t, bc_kv)):
                        psb = pbc.tile([128, 512], f32, tag="pbc")
                        mm(psb, ones_row, rct[:, sl], True, True)
                        bt = bcp.tile([128, 512], f32, name=f"bc{ty}_{tt}")
                        nc.vector.tensor_copy(bt[:], psb[:])
                        dst.append(bt)

            # projections with normalization fused into the PSUM->SBUF step
            with tc.tile_pool(name="p1ps", bufs=3, space="PSUM") as p1ps, \
                 tc.tile_pool(name="p1psv", bufs=2, space="PSUM") as p1psv:
                for tt in range(4):
                    sl = slice(tt * 512, (tt + 1) * 512)
                    for m in range(3):
                        ps = p1ps.tile([128, 512], f32, tag="p1ps")
                        for cc in range(12):
                            mm(ps, wqb_sb[:, cc, m * 128:(m + 1) * 128],
                               hq_all[:, cc, sl], cc == 0, cc == 11)
                        nc.vector.tensor_tensor(qdst[m][:, sl], ps[:],
                                                bc_hq[tt][:], OP.mult)
                    for kh in range(2):
                        ps = p1ps.tile([128, 512], f32, tag="p1ps")
                        for cc in range(4):
                            mm(ps, wkvb_sb[:, cc, kh * 128:(kh + 1) * 128],
                               kv_all[:, cc, sl], cc == 0, cc == 3)
                        nc.vector.tensor_tensor(kn[kh][:, sl], ps[:],
                                                bc_kv[tt][:], OP.mult)
                for tb in range(16):
                    ps = p1psv.tile([128, 256], f32, tag="p1psv")
                    for cc in range(4):
                        mm(ps, kv_all[:, cc, tb * 128:(tb + 1) * 128],
                           wkvb_sb[:, cc, 256:512], cc == 0, cc == 3)
                        # lhsT = latent chunk [c,t], rhs = v cols of wkv_b'^T
                    nc.vector.tensor_scalar_mul(vt[tb][:], ps[:],
                                                rckv_col[:, tb:tb + 1])

                # rope on q (both heads share qt1: rows 0:64 h0, 64:128 h1)
                tmp = prj.tile([128, S], b16)
                for b in (0, 64):
                    nc.sync.dma_start(tmp[b:b + 32], qt1[b + 32:b + 64])
                    nc.sync.dma_start(tmp[b + 32:b + 64], qt1[b:b + 32])
                nc.vector.tensor_tensor(qt1r[:], qt1[:], cos2_sb[:], OP.mult)
                nc.vector.tensor_tensor(tmp[:], tmp[:], sin2_sb[:], OP.mult)
                nc.vector.tensor_tensor(qt1r[:], qt1r[:], tmp[:], OP.add)
                # h1 rope rows to base-0 tile (rows 64: zero; kpe rows 64: 0)
                nc.sync.dma_start(qr1[0:64, :], qt1r[64:128])

        # wo weights prefetch (overlaps attention)
        nc.sync.dma_start(wot_sb[:], woT.ap().rearrange("o p s -> p o s"))

        # attention + split AllGather + column-parallel wo
        with tc.tile_pool(name="apss", bufs=3, space="PSUM") as apss, \
             tc.tile_pool(name="apsx", bufs=2, space="PSUM") as apsx, \
             tc.tile_pool(name="apsd", bufs=1, space="PSUM") as apsd, \
             tc.tile_pool(name="apsb", bufs=1, space="PSUM") as apsb, \
             tc.tile_pool(name="wops", bufs=1, space="PSUM") as wops, \
             tc.tile_pool(name="aex", bufs=3) as aexp, \
             tc.tile_pool(name="asm", bufs=2) as asmp, \
             tc.tile_pool(name="amk", bufs=2) as amkp, \
             tc.tile_pool(name="wop", bufs=1) as wop, \
             tc.tile_pool(name="woot", bufs=3) as wootp:

            LOOKAHEAD = 2

            def attend(h, xh):
                qn_h = qn0 if h == 0 else qn1
                qr_h = qt1r if h == 0 else qr1
                for sb in range(4):
                    sl = slice(sb * 512, (sb + 1) * 512)
                    psx = apsx.tile([128, 512], f32, tag="apsx")
                    psd = apsd.tile([1, 512], f32, tag="apsd")
                    exq = []

                    def consume(tb, ex):
                        mm(psx, vt[tb][:, h * 128:(h + 1) * 128], ex,
                           tb == 0, tb == 15)
                        mm(psd, ones_col, ex, tb == 0, tb == 15)

                    for tb in range(16):
                        tsl = slice(tb * 128, (tb + 1) * 128)
                        pss = apss.tile([128, 512], f32, tag="apss")
                        mm(pss, kn[h][:, tsl], qn_h[:, sl], True, False)
                        mm(pss, kpe_sb[:, tsl], qr_h[:, sl], False, True)
                        if has_mask:
                            mk = amkp.tile([128, 512], b16, tag="amk")
                            nc.sync.dma_start(
                                mk[:], maskT.ap()[tsl, sl])
                            nc.vector.tensor_tensor(pss[:], pss[:], mk[:],
                                                    OP.add)
                        ex = aexp.tile([128, 512], b16, tag="aex")
                        nc.scalar.activation(ex[:], pss[:], AF.Exp)
                        exq.append((tb, ex))
                        if len(exq) > LOOKAHEAD:
                            consume(*exq.pop(0))
                    for item in exq:
                        consume(*item)
                    rd = asmp.tile([1, 512], f32, tag="rd")
                    nc.vector.reciprocal(rd[:], psd[:])
                    psb2 = apsb.tile([128, 512], f32, tag="apsb")
                    mm(psb2, ones_row, rd, True, True)
                    rdb = asmp.tile([128, 512], f32, tag="rdb")
                    nc.vector.tensor_copy(rdb[:], psb2[:])
                    nc.vector.tensor_tensor(xh[:, sl], psx[:], rdb[:],
                                            OP.mult)

            attend(0, xh0)
            nc.sync.dma_start(bounce2a.ap(), xh0[:])
            nc.gpsimd.collective_compute(
                "AllGather", OP.bypass, replica_groups=RG,
                ins=[bounce2a.ap().opt()], outs=[gath2a.ap().opt()])

            # head-1 attention runs while gath2a lands + wo half-accumulates
            attend(1, xh1)

            xe_a = wop.tile([128, 8, S], b16)
            for r in range(NCORES):
                nc.sync.dma_start(xe_a[:, r, :], gath2a.ap()[r])
            opart = [wop.tile([128, SSH], f32, name=f"op{st}")
                     for st in range(16)]
            for st in range(16):
                pso = wops.tile([128, SSH], f32, tag="wops")
                for r in range(NCORES):
                    mm(pso, xe_a[:, r, st * 128:(st + 1) * 128],
                       wot_sb[:, 2 * r, :], r == 0, r == 7)
                nc.vector.tensor_copy(opart[st][:], pso[:])

            nc.sync.dma_start(bounce2b.ap(), xh1[:])
            nc.gpsimd.collective_compute(
                "AllGather", OP.bypass, replica_groups=RG,
                ins=[bounce2b.ap().opt()], outs=[gath2b.ap().opt()])

            xe_b = wop.tile([128, 8, S], b16)
            for r in range(NCORES):
                nc.sync.dma_start(xe_b[:, r, :], gath2b.ap()[r])
            for st in range(16):
                pso = wops.tile([128, SSH], f32, tag="wops")
                for r in range(NCORES):
                    mm(pso, xe_b[:, r, st * 128:(st + 1) * 128],
                       wot_sb[:, 2 * r + 1, :], r == 0, r == 7)
                ot = wootp.tile([128, SSH], f32, tag="ot")
                nc.vector.tensor_tensor(ot[:], pso[:], opart[st][:], OP.add)
                nc.sync.dma_start(out.ap()[st * 128:(st + 1) * 128, :], ot[:])

    nc.compile()
    return nc


def _prep_inputs(hidden_states, cos, sin, attn_mask, wq_a, q_norm_w, wq_b,
                 wkv_a, kv_norm_w, wkv_b, wo, has_mask):
    import ml_dtypes
    bf = ml_dtypes.bfloat16

    def c(x):
        return np.ascontiguousarray(x.astype(bf))

    hid = np.asarray(hidden_states, np.float32)[0]          # [S, H]
    hidT = c(hid.T.reshape(16, 128, S))                     # [H, S]
    A_T = np.vstack([np.asarray(wq_a, np.float32),
                     np.asarray(wkv_a, np.float32)]).T      # [H, CTOT]
    a_pe = c(A_T[:, 2048:2112].reshape(16, 128, 64))

    cosT = np.asarray(cos, np.float32).T                    # [64, S]
    sinT = np.asarray(sin, np.float32).T
    sinTs = sinT.copy()
    sinTs[0:32] *= -1.0
    cosT2 = c(np.concatenate([cosT, cosT], 0))              # [128, S]
    sinT2s = c(np.concatenate([sinTs, sinTs], 0))

    wqb = np.asarray(wq_b, np.float32) * np.asarray(q_norm_w, np.float32)[None]
    wqb = wqb * SOFTMAX_SCALE
    wkvb = (np.asarray(wkv_b, np.float32)
            * np.asarray(kv_norm_w, np.float32)[None])
    woT_full = np.asarray(wo, np.float32).T                 # [NH*DV, H]

    qperm = np.r_[0:128, 128:192, 320:384, 192:320]
    kvperm = np.r_[0:128, 256:384, 128:256, 384:512]

    selT = np.zeros((16, 2), np.float32)
    for p in range(0, 16, 2):
        if p < 12:
            selT[p, 0] = 1.0 / Q_LORA
        else:
            selT[p, 1] = 1.0 / KV_LORA
    selH = np.ascontiguousarray(selT[:, 0:1])
    selC = np.ascontiguousarray(selT[:, 1:2])

    in_maps = []
    for r in range(NCORES):
        m = {
            "hidT": hidT,
            "a_own": c(A_T[:, r * 256:(r + 1) * 256].reshape(16, 128, 256)),
            "a_pe": a_pe,
            "cosT2": cosT2,
            "sinT2s": sinT2s,
            "wqbT": c(wqb[r * 384:(r + 1) * 384].T[:, qperm]
                      .reshape(12, 128, 384)),
            "wkvbT": c(wkvb[r * 512:(r + 1) * 512].T[:, kvperm]
                       .reshape(4, 128, 512)),
            "woT": c(woT_full[:, r * SSH:(r + 1) * SSH].reshape(16, 128, SSH)),
            "ones_a": np.ones((128, 1), bf),
            "ones_bf": np.ones((1, 128), np.float32),
            "selH": c(selH),
            "selC": c(selC),
        }
        if has_mask:
            m["maskT"] = c(np.asarray(attn_mask, np.float32).T)
        in_maps.append(m)
    return in_maps


def kernel(**inputs):
    from concourse.bass_utils import run_bass_kernel_spmd

    has_mask = bool(np.any(np.asarray(inputs["attn_mask"])))
    if has_mask not in _CACHE:
        _CACHE[has_mask] = _build(has_mask)
    nc = _CACHE[has_mask]

    in_maps = _prep_inputs(has_mask=has_mask, **inputs)
    res = run_bass_kernel_spmd(nc, in_maps, list(range(NCORES))).results
    full = np.concatenate([res[r]["out"] for r in range(NCORES)], axis=1)
    return full.reshape(B, S, H).astype(np.float32)


# revision 21
# speedup vs baseline: 1.5162x; 1.0570x over previous
"""MLA (DeepSeek-style multi-head latent attention) Bass kernel for 8 trn2 NeuronCores.

Sharding: tensor-parallel over heads (2 heads/core) for the big projections +
attention. The low-rank A-projections are CHANNEL-sharded (each core computes
256 of the 2048 hq+kv latent channels for all 2048 tokens) so the AllGathered
latents read back as contiguous 4KB runs; k_pe (64 rope channels) is computed
redundantly on every core, skipping it in the collective. RMS normalization
happens after the gather: per-core partial sum-of-squares rides the AllGather
as one extra bf16 row per channel block, and the rsqrt factors are folded into
the projection PSUM->SBUF copies (free-dim broadcast for q/k, per-partition
tensor_scalar for v). The output projection is column-parallel with a split
AllGather: head 0's attention output gathers and partially accumulates into
wo while head 1's attention still runs.

All tensors bf16 (fp32 PSUM accumulation); exp softmax without max-subtract.
Host-side (free) preprocessing: bf16 casts, weight transposes/permutations,
folding q_norm_w/kv_norm_w and SOFTMAX_SCALE into wq_b/wkv_b, rope sign folds.
"""

import math
import sys

import numpy as np

for _p in ("/opt/trn_rl_repo", "/root/.axon_site/_ro/trn_rl_repo"):
    if _p not in sys.path:
        sys.path.append(_p)

B, S, H = 1, 2048, 2048
NH = 16
Q_LORA, KV_LORA = 1536, 512
D_NOPE, D_ROPE, D_V = 128, 64, 128
D_QK = D_NOPE + D_ROPE
ROPE_FACTOR, MSCALE = 4.0, 1.0
SOFTMAX_SCALE = D_QK ** -0.5 * (0.1 * MSCALE * math.log(ROPE_FACTOR) + 1.0) ** 2
EPS = 1e-6

NCORES = 8
SSH = S // NCORES          # 256 output channels per core (wo column-parallel)
CTOT = Q_LORA + KV_LORA + D_ROPE   # 2112 latent channels

_CACHE = {}


def _build(has_mask: bool):
    import concourse.bacc as bacc
    import concourse.mybir as mybir
    import concourse.tile as tile

    f32 = mybir.dt.float32
    b16 = mybir.dt.bfloat16
    AF = mybir.ActivationFunctionType
    OP = mybir.AluOpType

    nc = bacc.Bacc("TRN2", target_bir_lowering=False, debug=False,
                   num_devices=NCORES)

    hidT = nc.dram_tensor("hidT", [16, 128, S], b16, kind="ExternalInput")
    a_own = nc.dram_tensor("a_own", [16, 128, 256], b16, kind="ExternalInput")
    a_pe = nc.dram_tensor("a_pe", [16, 128, 64], b16, kind="ExternalInput")
    cosT2 = nc.dram_tensor("cosT2", [128, S], b16, kind="ExternalInput")
    sinT2s = nc.dram_tensor("sinT2s", [128, S], b16, kind="ExternalInput")
    wqbT = nc.dram_tensor("wqbT", [12, 128, 384], b16, kind="ExternalInput")
    wkvbT = nc.dram_tensor("wkvbT", [4, 128, 512], b16, kind="ExternalInput")
    woT = nc.dram_tensor("woT", [16, 128, SSH], b16, kind="ExternalInput")
    ones_a = nc.dram_tensor("ones_a", [128, 1], b16, kind="ExternalInput")
    ones_bf = nc.dram_tensor("ones_bf", [1, 128], f32, kind="ExternalInput")
    selH_d = nc.dram_tensor("selH", [16, 1], b16, kind="ExternalInput")
    selC_d = nc.dram_tensor("selC", [16, 1], b16, kind="ExternalInput")
    if has_mask:
        maskT = nc.dram_tensor("maskT", [S, S], b16, kind="ExternalInput")
    out = nc.dram_tensor("out", [S, SSH], f32, kind="ExternalOutput")

    bounce1 = [nc.dram_tensor(f"bounce1{i}", [2, 129, S // 2], b16)
               for i in range(2)]
    gath1 = [nc.dram_tensor(f"gath1{i}", [NCORES, 2, 129, S // 2], b16,
                            addr_space="Shared") for i in range(2)]
    bounce2a = nc.dram_tensor("bounce2a", [128, S], b16)
    gath2a = nc.dram_tensor("gath2a", [NCORES, 128, S], b16,
                            addr_space="Shared")
    bounce2b = [nc.dram_tensor(f"bounce2b{i}", [128, S // 2], b16)
                for i in range(2)]
    gath2b = [nc.dram_tensor(f"gath2b{i}", [NCORES, 128, S // 2], b16,
                             addr_space="Shared") for i in range(2)]

    RG = [list(range(NCORES))]

    def mm(ps, lhsT, rhs, start, stop):
        nc.tensor.matmul(ps, lhsT, rhs, start=start, stop=stop)

    from contextlib import ExitStack
    with tile.TileContext(nc) as tc, ExitStack() as _st:
        constp = _st.enter_context(tc.tile_pool(name="const", bufs=1))
        ones_col = constp.tile([128, 1], b16)
        nc.sync.dma_start(ones_col[:], ones_a.ap())
        ones_row = constp.tile([1, 128], f32)
        nc.sync.dma_start(ones_row[:], ones_bf.ap())
        selh_sb = constp.tile([16, 1], b16)
        nc.sync.dma_start(selh_sb[:], selH_d.ap())
        selc_sb = constp.tile([16, 1], b16)
        nc.sync.dma_start(selc_sb[:], selC_d.ap())
        eps1 = constp.tile([1, 1], f32)
        nc.any.memset(eps1[:], EPS)
        eps_col = constp.tile([128, 1], f32)
        nc.any.memset(eps_col[:], EPS)

        # stage-1 weight tiles (DMAs issued after stage-0's loads)
        s1wp = _st.enter_context(tc.tile_pool(name="s1w", bufs=1))
        wqb_sb = s1wp.tile([128, 12, 384], b16)
        wkvb_sb = s1wp.tile([128, 4, 512], b16)
        cos2_sb = s1wp.tile([128, S], b16)
        sin2_sb = s1wp.tile([128, S], b16)
        wot_sb = s1wp.tile([128, 16, SSH], b16)   # DMA issued before attention

        # attention-lifetime pool (also covers the wo epilogue)
        attp = _st.enter_context(tc.tile_pool(name="att", bufs=1))
        kpe_sb = attp.tile([128, S], b16)
        nc.any.memset(kpe_sb[64:128, :], 0.0)

        # ---------------- stage 0: own 256 latent channels for all tokens
        with tc.tile_pool(name="s0", bufs=1) as s0p, \
             tc.tile_pool(name="s0ps", bufs=3, space="PSUM") as s0ps, \
             tc.tile_pool(name="s0ss", bufs=2, space="PSUM") as s0ssp, \
             tc.tile_pool(name="s0pe", bufs=2, space="PSUM") as s0pe, \
             tc.tile_pool(name="s0sq", bufs=3) as s0sqp:
            a_sb = s0p.tile([128, 16, 256], b16)
            nc.sync.dma_start(a_sb[:], a_own.ap().rearrange("o p c -> p o c"))
            ape_sb = s0p.tile([128, 16, 64], b16)
            nc.sync.dma_start(ape_sb[:], a_pe.ap().rearrange("o p c -> p o c"))
            hid_sb = s0p.tile([128, 16, S], b16)
            for g in range(8):
                nc.sync.dma_start(
                    hid_sb[:, g * 2:(g + 1) * 2, :],
                    hidT.ap()[g * 2:(g + 1) * 2].rearrange("o p s -> p o s"))
            # stage-1 weights load behind stage-0's operands
            nc.sync.dma_start(cos2_sb[:], cosT2.ap())
            nc.sync.dma_start(sin2_sb[:], sinT2s.ap())
            nc.sync.dma_start(wqb_sb[:],
                              wqbT.ap().rearrange("o p d -> p o d"))
            nc.sync.dma_start(wkvb_sb[:],
                              wkvbT.ap().rearrange("o p d -> p o d"))

            raw = s0p.tile([128, 2, S], b16)
            kpraw = s0p.tile([64, S], b16)
            ssb16 = s0p.tile([1, S], b16)
            zrow = s0p.tile([1, S], b16)
            nc.any.memset(zrow[:], 0.0)
            for tt in range(4):
                sl = slice(tt * 512, (tt + 1) * 512)
                ssp = s0ssp.tile([1, 512], f32, tag="ss")
                for ct in range(2):
                    ps = s0ps.tile([128, 512], f32, tag="s0ps")
                    for hb in range(16):
                        mm(ps, a_sb[:, hb, ct * 128:(ct + 1) * 128],
                           hid_sb[:, hb, sl], hb == 0, hb == 15)
                    nc.vector.tensor_copy(raw[:, ct, sl], ps[:])
                    sq = s0sqp.tile([128, 512], b16, tag="s0sq")
                    nc.scalar.activation(sq[:], ps[:], AF.Square)
                    mm(ssp, ones_col, sq, ct == 0, ct == 1)
                nc.scalar.copy(ssb16[:, sl], ssp[:])
                kp = s0pe.tile([64, 512], f32, tag="kpe")
                for hb in range(16):
                    mm(kp, ape_sb[:, hb, :], hid_sb[:, hb, sl],
                       hb == 0, hb == 15)
                nc.vector.tensor_copy(kpraw[:, sl], kp[:])

            # ship each token half as soon as its latents are done
            for i in range(2):
                hs = slice(i * 1024, (i + 1) * 1024)
                nc.sync.dma_start(
                    bounce1[i].ap()[:, 0:128, :].rearrange("o p s -> p o s"),
                    raw[:, :, hs])
                nc.sync.dma_start(bounce1[i].ap()[0, 128:129, :],
                                  ssb16[:, hs])
                nc.sync.dma_start(bounce1[i].ap()[1, 128:129, :],
                                  zrow[:, 0:1024])
                nc.gpsimd.collective_compute(
                    "AllGather", OP.bypass, replica_groups=RG,
                    ins=[bounce1[i].ap().opt()], outs=[gath1[i].ap().opt()])

            # k_pe rope (redundant on every core; not in the collective)
            t1 = s0p.tile([64, S], b16)
            nc.vector.tensor_tensor(t1[:], kpraw[:], cos2_sb[0:64, :], OP.mult)
            rsw = s0p.tile([64, S], b16)
            nc.sync.dma_start(rsw[0:32], kpraw[32:64])
            nc.sync.dma_start(rsw[32:64], kpraw[0:32])
            nc.vector.tensor_tensor(rsw[:], rsw[:], sin2_sb[0:64, :], OP.mult)
            nc.vector.tensor_tensor(kpe_sb[0:64, :], t1[:], rsw[:], OP.add)

        # ---------------- stage 1: per-head projections + attention
        qn0 = attp.tile([128, S], b16)
        qt1 = attp.tile([128, S], b16)
        qn1 = attp.tile([128, S], b16)
        qdst = (qn0, qt1, qn1)
        kn0 = attp.tile([128, S], b16)
        kn1 = attp.tile([128, S], b16)
        kn = (kn0, kn1)
        vt = [attp.tile([128, 256], b16, name=f"vt{tb}") for tb in range(16)]
        qt1r = attp.tile([128, S], b16)
        qr1 = attp.tile([128, S], b16)
        nc.any.memset(qr1[64:128, :], 0.0)
        xh0 = attp.tile([128, S], b16)
        xh1 = attp.tile([128, S], b16)

        with tc.tile_pool(name="proj", bufs=1) as prj, \
             tc.tile_pool(name="bcp", bufs=1) as bcp:
            # gathered reads: all contiguous 4KB runs
            partials = prj.tile([16, S], b16)
            hq_all = prj.tile([128, 12, S], b16)
            kv_all = prj.tile([128, 4, S], b16)
            for i in range(2):
                hs = slice(i * 1024, (i + 1) * 1024)
                nc.sync.dma_start(
                    partials[:, hs],
                    gath1[i].ap()[:, :, 128, :].rearrange("a b s -> (a b) s"))
                for r in range(6):
                    nc.sync.dma_start(
                        hq_all[:, 2 * r:2 * r + 2, hs],
                        gath1[i].ap()[r, :, 0:128, :]
                        .rearrange("o p s -> p o s"))
                for r in (6, 7):
                    nc.sync.dma_start(
                        kv_all[:, 2 * (r - 6):2 * (r - 6) + 2, hs],
                        gath1[i].ap()[r, :, 0:128, :]
                        .rearrange("o p s -> p o s"))

            # rms factors: [1, S] free-layout (q/k) + [128, 16] col-layout (v)
            sq_hq = prj.tile([1, S], f32)
            sq_kv = prj.tile([1, S], f32)
            rc_hq_t = prj.tile([1, S], f32)
            rc_kv_t = prj.tile([1, S], f32)
            sqcol = prj.tile([128, 16], f32)
            rckv_col = prj.tile([128, 16], f32)
            bc_hq, bc_kv = [], []
            with tc.tile_pool(name="pfac", bufs=2, space="PSUM") as pfac, \
                 tc.tile_pool(name="pbc", bufs=2, space="PSUM") as pbc:
                for tt in range(4):
                    sl = slice(tt * 512, (tt + 1) * 512)
                    for selt, sqt in ((selh_sb, sq_hq), (selc_sb, sq_kv)):
                        ps2 = pfac.tile([1, 512], f32, tag="ps2")
                        mm(ps2, selt, partials[:, sl], True, True)
                        nc.scalar.activation(sqt[:, sl], ps2[:], AF.Sqrt,
                                             bias=eps1[:])
                nc.vector.reciprocal(rc_hq_t[:], sq_hq[:])
                nc.vector.reciprocal(rc_kv_t[:], sq_kv[:])
                pscol = pfac.tile([128, 16], f32, tag="pscol")
                for tb in range(16):
                    mm(pscol[:, tb:tb + 1],
                       partials[:, tb * 128:(tb + 1) * 128],
                       selc_sb, True, True)
                nc.scalar.activation(sqcol[:], pscol[:], AF.Sqrt,
                                     bias=eps_col[:])
                nc.vector.reciprocal(rckv_col[:], sqcol[:])

                # broadcast rsqrt factors to 128 partitions, per 512-tok chunk
                for tt in range(4):
                    sl = slice(tt * 512, (tt + 1) * 512)
                    for ty, rct, dst in ((0, rc_hq_t, bc_hq),
                                         (1, rc_kv_t, bc_kv)):
                        psb = pbc.tile([128, 512], f32, tag="pbc")
                        mm(psb, ones_row, rct[:, sl], True, True)
                        bt = bcp.tile([128, 512], f32, name=f"bc{ty}_{tt}")
                        nc.vector.tensor_copy(bt[:], psb[:])
                        dst.append(bt)

            # projections with normalization fused into the PSUM->SBUF step
            with tc.tile_pool(name="p1ps", bufs=3, space="PSUM") as p1ps, \
                 tc.tile_pool(name="p1psv", bufs=2, space="PSUM") as p1psv:
                for tt in range(4):
                    sl = slice(tt * 512, (tt + 1) * 512)
                    for m in range(3):
                        ps = p1ps.tile([128, 512], f32, tag="p1ps")
                        for cc in range(12):
                            mm(ps, wqb_sb[:, cc, m * 128:(m + 1) * 128],
                               hq_all[:, cc, sl], cc == 0, cc == 11)
                        nc.vector.tensor_tensor(qdst[m][:, sl], ps[:],
                                                bc_hq[tt][:], OP.mult)
                    for kh in range(2):
                        ps = p1ps.tile([128, 512], f32, tag="p1ps")
                        for cc in range(4):
                            mm(ps, wkvb_sb[:, cc, kh * 128:(kh + 1) * 128],
                               kv_all[:, cc, sl], cc == 0, cc == 3)
                        nc.vector.tensor_tensor(kn[kh][:, sl], ps[:],
                                                bc_kv[tt][:], OP.mult)
                for tb in range(16):
                    ps = p1psv.tile([128, 256], f32, tag="p1psv")
                    for cc in range(4):
                        mm(ps, kv_all[:, cc, tb * 128:(tb + 1) * 128],
                           wkvb_sb[:, cc, 256:512], cc == 0, cc == 3)
                        # lhsT = latent chunk [c,t], rhs = v cols of wkv_b'^T
                    nc.vector.tensor_scalar_mul(vt[tb][:], ps[:],
                                                rckv_col[:, tb:tb + 1])

                # rope on q (both heads share qt1: rows 0:64 h0, 64:128 h1)
                tmp = prj.tile([128, S], b16)
                for b in (0, 64):
                    nc.sync.dma_start(tmp[b:b + 32], qt1[b + 32:b + 64])
                    nc.sync.dma_start(tmp[b + 32:b + 64], qt1[b:b + 32])
                nc.vector.tensor_tensor(qt1r[:], qt1[:], cos2_sb[:], OP.mult)
                nc.vector.tensor_tensor(tmp[:], tmp[:], sin2_sb[:], OP.mult)
                nc.vector.tensor_tensor(qt1r[:], qt1r[:], tmp[:], OP.add)
                # h1 rope rows to base-0 tile (rows 64: zero; kpe rows 64: 0)
                nc.sync.dma_start(qr1[0:64, :], qt1r[64:128])

        # wo weights prefetch (overlaps attention)
        nc.sync.dma_start(wot_sb[:], woT.ap().rearrange("o p s -> p o s"))

        # attention + split AllGather + column-parallel wo
        with tc.tile_pool(name="apss", bufs=4, space="PSUM") as apss, \
             tc.tile_pool(name="apsx", bufs=2, space="PSUM") as apsx, \
             tc.tile_pool(name="apsd", bufs=1, space="PSUM") as apsd, \
             tc.tile_pool(name="wops", bufs=1, space="PSUM") as wops, \
             tc.tile_pool(name="aex", bufs=4) as aexp, \
             tc.tile_pool(name="asm", bufs=2) as asmp, \
             tc.tile_pool(name="amk", bufs=2) as amkp, \
             tc.tile_pool(name="wop", bufs=1) as wop, \
             tc.tile_pool(name="woot", bufs=3) as wootp:

            LOOKAHEAD = 3

            def attend(h, xh, ship=None):
                qn_h = qn0 if h == 0 else qn1
                qr_h = qt1r if h == 0 else qr1
                for sb in range(4):
                    sl = slice(sb * 512, (sb + 1) * 512)
                    psx = apsx.tile([128, 512], f32, tag="apsx")
                    psdt = apsd.tile([128, 512], f32, tag="dn")
                    psd = psdt[0:1, :]
                    exq = []

                    def consume(tb, ex):
                        mm(psx, vt[tb][:, h * 128:(h + 1) * 128], ex,
                           tb == 0, tb == 15)
                        mm(psd, ones_col, ex, tb == 0, tb == 15)

                    for tb in range(16):
                        tsl = slice(tb * 128, (tb + 1) * 128)
                        pss = apss.tile([128, 512], f32, tag="apss")
                        mm(pss, kn[h][:, tsl], qn_h[:, sl], True, False)
                        mm(pss, kpe_sb[:, tsl], qr_h[:, sl], False, True)
                        if has_mask:
                            mk = amkp.tile([128, 512], b16, tag="amk")
                            nc.sync.dma_start(
                                mk[:], maskT.ap()[tsl, sl])
                            nc.vector.tensor_tensor(pss[:], pss[:], mk[:],
                                                    OP.add)
                        ex = aexp.tile([128, 512], b16, tag="aex")
                        nc.scalar.activation(ex[:], pss[:], AF.Exp)
                        exq.append((tb, ex))
                        if len(exq) > LOOKAHEAD:
                            consume(*exq.pop(0))
                    for item in exq:
                        consume(*item)
                    rd = asmp.tile([1, 512], f32, tag="rd")
                    nc.vector.reciprocal(rd[:], psd[:])
                    psb2 = apsd.tile([128, 512], f32, tag="dn")
                    mm(psb2, ones_row, rd, True, True)
                    rdb = asmp.tile([128, 512], f32, tag="rdb")
                    nc.vector.tensor_copy(rdb[:], psb2[:])
                    nc.vector.tensor_tensor(xh[:, sl], psx[:], rdb[:],
                                            OP.mult)
                    if ship is not None:
                        ship(sb)

            attend(0, xh0)
            nc.sync.dma_start(bounce2a.ap(), xh0[:])
            nc.gpsimd.collective_compute(
                "AllGather", OP.bypass, replica_groups=RG,
                ins=[bounce2a.ap().opt()], outs=[gath2a.ap().opt()])

            # head-1 attention runs while gath2a lands + wo half-accumulates
            def ship_h1(sb):
                if sb in (1, 3):
                    i = sb // 2
                    nc.sync.dma_start(bounce2b[i].ap(),
                                      xh1[:, i * 1024:(i + 1) * 1024])
                    nc.gpsimd.collective_compute(
                        "AllGather", OP.bypass, replica_groups=RG,
                        ins=[bounce2b[i].ap().opt()],
                        outs=[gath2b[i].ap().opt()])

            attend(1, xh1, ship=ship_h1)

            xe_a = wop.tile([128, 8, S], b16)
            for r in range(NCORES):
                nc.sync.dma_start(xe_a[:, r, :], gath2a.ap()[r])
            opart = [wop.tile([128, SSH], f32, name=f"op{st}")
                     for st in range(16)]
            for st in range(16):
                pso = wops.tile([128, SSH], f32, tag="wops")
                for r in range(NCORES):
                    mm(pso, xe_a[:, r, st * 128:(st + 1) * 128],
                       wot_sb[:, 2 * r, :], r == 0, r == 7)
                nc.vector.tensor_copy(opart[st][:], pso[:])

            xe_b = wop.tile([128, 8, S], b16)
            for i in range(2):
                hs = slice(i * 1024, (i + 1) * 1024)
                for r in range(NCORES):
                    nc.sync.dma_start(xe_b[:, r, hs], gath2b[i].ap()[r])
                for st in range(i * 8, (i + 1) * 8):
                    pso = wops.tile([128, SSH], f32, tag="wops")
                    for r in range(NCORES):
                        mm(pso, xe_b[:, r, st * 128:(st + 1) * 128],
                           wot_sb[:, 2 * r + 1, :], r == 0, r == 7)
                    ot = wootp.tile([128, SSH], f32, tag="ot")
                    nc.vector.tensor_tensor(ot[:], pso[:], opart[st][:],
                                            OP.add)
                    nc.sync.dma_start(out.ap()[st * 128:(st + 1) * 128, :],
                                      ot[:])

    nc.compile()
    return nc


def _prep_inputs(hidden_states, cos, sin, attn_mask, wq_a, q_norm_w, wq_b,
                 wkv_a, kv_norm_w, wkv_b, wo, has_mask):
    import ml_dtypes
    bf = ml_dtypes.bfloat16

    def c(x):
        return np.ascontiguousarray(x.astype(bf))

    hid = np.asarray(hidden_states, np.float32)[0]          # [S, H]
    hidT = c(hid.T.reshape(16, 128, S))                     # [H, S]
    A_T = np.vstack([np.asarray(wq_a, np.float32),
                     np.asarray(wkv_a, np.float32)]).T      # [H, CTOT]
    a_pe = c(A_T[:, 2048:2112].reshape(16, 128, 64))

    cosT = np.asarray(cos, np.float32).T                    # [64, S]
    sinT = np.asarray(sin, np.float32).T
    sinTs = sinT.copy()
    sinTs[0:32] *= -1.0
    cosT2 = c(np.concatenate([cosT, cosT], 0))              # [128, S]
    sinT2s = c(np.concatenate([sinTs, sinTs], 0))

    wqb = np.asarray(wq_b, np.float32) * np.asarray(q_norm_w, np.float32)[None]
    wqb = wqb * SOFTMAX_SCALE
    wkvb = (np.asarray(wkv_b, np.float32)
            * np.asarray(kv_norm_w, np.float32)[None])
    woT_full = np.asarray(wo, np.float32).T                 # [NH*DV, H]

    qperm = np.r_[0:128, 128:192, 320:384, 192:320]
    kvperm = np.r_[0:128, 256:384, 128:256, 384:512]

    selT = np.zeros((16, 2), np.float32)
    for p in range(0, 16, 2):
        if p < 12:
            selT[p, 0] = 1.0 / Q_LORA
        else:
            selT[p, 1] = 1.0 / KV_LORA
    selH = np.ascontiguousarray(selT[:, 0:1])
    selC = np.ascontiguousarray(selT[:, 1:2])

    in_maps = []
    for r in range(NCORES):
        m = {
            "hidT": hidT,
            "a_own": c(A_T[:, r * 256:(r + 1) * 256].reshape(16, 128, 256)),
            "a_pe": a_pe,
            "cosT2": cosT2,
            "sinT2s": sinT2s,
            "wqbT": c(wqb[r * 384:(r + 1) * 384].T[:, qperm]
                      .reshape(12, 128, 384)),
            "wkvbT": c(wkvb[r * 512:(r + 1) * 512].T[:, kvperm]
                       .reshape(4, 128, 512)),
            "woT": c(woT_full[:, r * SSH:(r + 1) * SSH].reshape(16, 128, SSH)),
            "ones_a": np.ones((128, 1), bf),
            "ones_bf": np.ones((1, 128), np.float32),
            "selH": c(selH),
            "selC": c(selC),
        }
        if has_mask:
            m["maskT"] = c(np.asarray(attn_mask, np.float32).T)
        in_maps.append(m)
    return in_maps


def kernel(**inputs):
    from concourse.bass_utils import run_bass_kernel_spmd

    has_mask = bool(np.any(np.asarray(inputs["attn_mask"])))
    if has_mask not in _CACHE:
        _CACHE[has_mask] = _build(has_mask)
    nc = _CACHE[has_mask]

    in_maps = _prep_inputs(has_mask=has_mask, **inputs)
    res = run_bass_kernel_spmd(nc, in_maps, list(range(NCORES))).results
    full = np.concatenate([res[r]["out"] for r in range(NCORES)], axis=1)
    return full.reshape(B, S, H).astype(np.float32)


# revision 22
# speedup vs baseline: 1.5385x; 1.0147x over previous
"""MLA (DeepSeek-style multi-head latent attention) Bass kernel for 8 trn2 NeuronCores.

Sharding: tensor-parallel over heads (2 heads/core) for the big projections +
attention. The low-rank A-projections are CHANNEL-sharded (each core computes
256 of the 2048 hq+kv latent channels for all 2048 tokens) so the AllGathered
latents read back as contiguous 4KB runs; k_pe (64 rope channels) is computed
redundantly on every core, skipping it in the collective. RMS normalization
happens after the gather: per-core partial sum-of-squares rides the AllGather
as one extra bf16 row per channel block, and the rsqrt factors are folded into
the projection PSUM->SBUF copies (free-dim broadcast for q/k, per-partition
tensor_scalar for v). The output projection is column-parallel with a split
AllGather: head 0's attention output gathers and partially accumulates into
wo while head 1's attention still runs.

All tensors bf16 (fp32 PSUM accumulation); exp softmax without max-subtract.
Host-side (free) preprocessing: bf16 casts, weight transposes/permutations,
folding q_norm_w/kv_norm_w and SOFTMAX_SCALE into wq_b/wkv_b, rope sign folds.
"""

import math
import sys

import numpy as np

for _p in ("/opt/trn_rl_repo", "/root/.axon_site/_ro/trn_rl_repo"):
    if _p not in sys.path:
        sys.path.append(_p)

B, S, H = 1, 2048, 2048
NH = 16
Q_LORA, KV_LORA = 1536, 512
D_NOPE, D_ROPE, D_V = 128, 64, 128
D_QK = D_NOPE + D_ROPE
ROPE_FACTOR, MSCALE = 4.0, 1.0
SOFTMAX_SCALE = D_QK ** -0.5 * (0.1 * MSCALE * math.log(ROPE_FACTOR) + 1.0) ** 2
EPS = 1e-6

NCORES = 8
SSH = S // NCORES          # 256 output channels per core (wo column-parallel)
CTOT = Q_LORA + KV_LORA + D_ROPE   # 2112 latent channels

_CACHE = {}


def _build(has_mask: bool):
    import concourse.bacc as bacc
    import concourse.mybir as mybir
    import concourse.tile as tile

    f32 = mybir.dt.float32
    b16 = mybir.dt.bfloat16
    AF = mybir.ActivationFunctionType
    OP = mybir.AluOpType

    nc = bacc.Bacc("TRN2", target_bir_lowering=False, debug=False,
                   num_devices=NCORES)

    hidT = nc.dram_tensor("hidT", [16, 128, S], b16, kind="ExternalInput")
    a_own = nc.dram_tensor("a_own", [16, 128, 256], b16, kind="ExternalInput")
    a_pe = nc.dram_tensor("a_pe", [16, 128, 64], b16, kind="ExternalInput")
    cosT2 = nc.dram_tensor("cosT2", [128, S], b16, kind="ExternalInput")
    sinT2s = nc.dram_tensor("sinT2s", [128, S], b16, kind="ExternalInput")
    wqbT = nc.dram_tensor("wqbT", [12, 128, 384], b16, kind="ExternalInput")
    wkvbT = nc.dram_tensor("wkvbT", [4, 128, 512], b16, kind="ExternalInput")
    woT = nc.dram_tensor("woT", [16, 128, SSH], b16, kind="ExternalInput")
    ones_a = nc.dram_tensor("ones_a", [128, 1], b16, kind="ExternalInput")
    ones_bf = nc.dram_tensor("ones_bf", [1, 128], f32, kind="ExternalInput")
    selH_d = nc.dram_tensor("selH", [16, 1], b16, kind="ExternalInput")
    selC_d = nc.dram_tensor("selC", [16, 1], b16, kind="ExternalInput")
    if has_mask:
        maskT = nc.dram_tensor("maskT", [S, S], b16, kind="ExternalInput")
    out = nc.dram_tensor("out", [S, SSH], f32, kind="ExternalOutput")

    bounce1 = nc.dram_tensor("bounce1", [2, 129, S], b16)
    gath1 = nc.dram_tensor("gath1", [NCORES, 2, 129, S], b16,
                           addr_space="Shared")
    bounce2a = nc.dram_tensor("bounce2a", [128, S], b16)
    gath2a = nc.dram_tensor("gath2a", [NCORES, 128, S], b16,
                            addr_space="Shared")
    B2SPLIT = 1536
    bounce2b = [nc.dram_tensor("bounce2b0", [128, B2SPLIT], b16),
                nc.dram_tensor("bounce2b1", [128, S - B2SPLIT], b16)]
    gath2b = [nc.dram_tensor("gath2b0", [NCORES, 128, B2SPLIT], b16,
                             addr_space="Shared"),
              nc.dram_tensor("gath2b1", [NCORES, 128, S - B2SPLIT], b16,
                             addr_space="Shared")]

    RG = [list(range(NCORES))]

    def mm(ps, lhsT, rhs, start, stop):
        nc.tensor.matmul(ps, lhsT, rhs, start=start, stop=stop)

    from contextlib import ExitStack
    with tile.TileContext(nc) as tc, ExitStack() as _st:
        constp = _st.enter_context(tc.tile_pool(name="const", bufs=1))
        ones_col = constp.tile([128, 1], b16)
        nc.sync.dma_start(ones_col[:], ones_a.ap())
        ones_row = constp.tile([1, 128], f32)
        nc.sync.dma_start(ones_row[:], ones_bf.ap())
        selh_sb = constp.tile([16, 1], b16)
        nc.sync.dma_start(selh_sb[:], selH_d.ap())
        selc_sb = constp.tile([16, 1], b16)
        nc.sync.dma_start(selc_sb[:], selC_d.ap())
        eps1 = constp.tile([1, 1], f32)
        nc.any.memset(eps1[:], EPS)
        eps_col = constp.tile([128, 1], f32)
        nc.any.memset(eps_col[:], EPS)

        # stage-1 weight tiles (DMAs issued after stage-0's loads)
        s1wp = _st.enter_context(tc.tile_pool(name="s1w", bufs=1))
        wqb_sb = s1wp.tile([128, 12, 384], b16)
        wkvb_sb = s1wp.tile([128, 4, 512], b16)
        cos2_sb = s1wp.tile([128, S], b16)
        sin2_sb = s1wp.tile([128, S], b16)
        wot_sb = s1wp.tile([128, 16, SSH], b16)   # DMA issued before attention

        # attention-lifetime pool (also covers the wo epilogue)
        attp = _st.enter_context(tc.tile_pool(name="att", bufs=1))
        kpe_sb = attp.tile([128, S], b16)
        nc.any.memset(kpe_sb[64:128, :], 0.0)

        # ---------------- stage 0: own 256 latent channels for all tokens
        with tc.tile_pool(name="s0", bufs=1) as s0p, \
             tc.tile_pool(name="s0ps", bufs=3, space="PSUM") as s0ps, \
             tc.tile_pool(name="s0ss", bufs=2, space="PSUM") as s0ssp, \
             tc.tile_pool(name="s0pe", bufs=2, space="PSUM") as s0pe, \
             tc.tile_pool(name="s0sq", bufs=3) as s0sqp:
            a_sb = s0p.tile([128, 16, 256], b16)
            hid_sb = s0p.tile([128, 16, S], b16)
            for g in range(8):
                gs = slice(g * 2, (g + 1) * 2)
                nc.sync.dma_start(
                    a_sb[:, gs, :],
                    a_own.ap()[gs].rearrange("o p c -> p o c"))
                nc.sync.dma_start(
                    hid_sb[:, gs, :],
                    hidT.ap()[gs].rearrange("o p s -> p o s"))
            ape_sb = s0p.tile([128, 16, 64], b16)
            nc.sync.dma_start(ape_sb[:], a_pe.ap().rearrange("o p c -> p o c"))
            # stage-1 weights load behind stage-0's operands
            nc.sync.dma_start(cos2_sb[:], cosT2.ap())
            nc.sync.dma_start(sin2_sb[:], sinT2s.ap())
            nc.sync.dma_start(wqb_sb[:],
                              wqbT.ap().rearrange("o p d -> p o d"))
            nc.sync.dma_start(wkvb_sb[:],
                              wkvbT.ap().rearrange("o p d -> p o d"))

            raw = s0p.tile([128, 2, S], b16)
            kpraw = s0p.tile([64, S], b16)
            ssb16 = s0p.tile([1, S], b16)
            zrow = s0p.tile([1, S], b16)
            nc.any.memset(zrow[:], 0.0)
            for tt in range(4):
                sl = slice(tt * 512, (tt + 1) * 512)
                ssp = s0ssp.tile([1, 512], f32, tag="ss")
                for ct in range(2):
                    ps = s0ps.tile([128, 512], f32, tag="s0ps")
                    for hb in range(16):
                        mm(ps, a_sb[:, hb, ct * 128:(ct + 1) * 128],
                           hid_sb[:, hb, sl], hb == 0, hb == 15)
                    nc.vector.tensor_copy(raw[:, ct, sl], ps[:])
                    sq = s0sqp.tile([128, 512], b16, tag="s0sq")
                    nc.scalar.activation(sq[:], ps[:], AF.Square)
                    mm(ssp, ones_col, sq, ct == 0, ct == 1)
                nc.scalar.copy(ssb16[:, sl], ssp[:])

            # ship bounce pieces as each token chunk's latents finish;
            # the single collective then waits only on the last piece
            for i in range(4):
                hs = slice(i * 512, (i + 1) * 512)
                nc.sync.dma_start(
                    bounce1.ap()[:, 0:128, hs].rearrange("o p s -> p o s"),
                    raw[:, :, hs])
            nc.sync.dma_start(bounce1.ap()[0, 128:129, :], ssb16[:])
            nc.sync.dma_start(bounce1.ap()[1, 128:129, :], zrow[:])
            nc.gpsimd.collective_compute(
                "AllGather", OP.bypass, replica_groups=RG,
                ins=[bounce1.ap().opt()], outs=[gath1.ap().opt()])

            # k_pe matmuls after the ship: they fill the AllGather wait
            for tt in range(4):
                sl = slice(tt * 512, (tt + 1) * 512)
                kp = s0pe.tile([64, 512], f32, tag="kpe")
                for hb in range(16):
                    mm(kp, ape_sb[:, hb, :], hid_sb[:, hb, sl],
                       hb == 0, hb == 15)
                nc.vector.tensor_copy(kpraw[:, sl], kp[:])

            # k_pe rope (redundant on every core; not in the collective)
            t1 = s0p.tile([64, S], b16)
            nc.vector.tensor_tensor(t1[:], kpraw[:], cos2_sb[0:64, :], OP.mult)
            rsw = s0p.tile([64, S], b16)
            nc.sync.dma_start(rsw[0:32], kpraw[32:64])
            nc.sync.dma_start(rsw[32:64], kpraw[0:32])
            nc.vector.tensor_tensor(rsw[:], rsw[:], sin2_sb[0:64, :], OP.mult)
            nc.vector.tensor_tensor(kpe_sb[0:64, :], t1[:], rsw[:], OP.add)

        # ---------------- stage 1: per-head projections + attention
        qn0 = attp.tile([128, S], b16)
        qt1 = attp.tile([128, S], b16)
        qn1 = attp.tile([128, S], b16)
        qdst = (qn0, qt1, qn1)
        kn0 = attp.tile([128, S], b16)
        kn1 = attp.tile([128, S], b16)
        kn = (kn0, kn1)
        vt = [attp.tile([128, 256], b16, name=f"vt{tb}") for tb in range(16)]
        qt1r = attp.tile([128, S], b16)
        qr1 = attp.tile([128, S], b16)
        nc.any.memset(qr1[64:128, :], 0.0)
        xh0 = attp.tile([128, S], b16)
        xh1 = attp.tile([128, S], b16)

        with tc.tile_pool(name="proj", bufs=1) as prj, \
             tc.tile_pool(name="bcp", bufs=1) as bcp:
            # gathered reads: all contiguous 4KB runs
            partials = prj.tile([16, S], b16)
            hq_all = prj.tile([128, 12, S], b16)
            kv_all = prj.tile([128, 4, S], b16)
            nc.sync.dma_start(
                partials[:],
                gath1.ap()[:, :, 128, :].rearrange("a b s -> (a b) s"))
            for r in (6, 7):
                nc.sync.dma_start(
                    kv_all[:, 2 * (r - 6):2 * (r - 6) + 2, :],
                    gath1.ap()[r, :, 0:128, :].rearrange("o p s -> p o s"))
            for r in range(6):
                nc.sync.dma_start(
                    hq_all[:, 2 * r:2 * r + 2, :],
                    gath1.ap()[r, :, 0:128, :].rearrange("o p s -> p o s"))

            # rms factors: [1, S] free-layout (q/k) + [128, 16] col-layout (v)
            sq_hq = prj.tile([1, S], f32)
            sq_kv = prj.tile([1, S], f32)
            rc_hq_t = prj.tile([1, S], f32)
            rc_kv_t = prj.tile([1, S], f32)
            sqcol = prj.tile([128, 16], f32)
            rckv_col = prj.tile([128, 16], f32)
            bc_hq, bc_kv = [], []
            with tc.tile_pool(name="pfac", bufs=2, space="PSUM") as pfac, \
                 tc.tile_pool(name="pbc", bufs=2, space="PSUM") as pbc:
                for tt in range(4):
                    sl = slice(tt * 512, (tt + 1) * 512)
                    for selt, sqt in ((selh_sb, sq_hq), (selc_sb, sq_kv)):
                        ps2 = pfac.tile([1, 512], f32, tag="ps2")
                        mm(ps2, selt, partials[:, sl], True, True)
                        nc.scalar.activation(sqt[:, sl], ps2[:], AF.Sqrt,
                                             bias=eps1[:])
                nc.vector.reciprocal(rc_hq_t[:], sq_hq[:])
                nc.vector.reciprocal(rc_kv_t[:], sq_kv[:])
                pscol = pfac.tile([128, 16], f32, tag="pscol")
                for tb in range(16):
                    mm(pscol[:, tb:tb + 1],
                       partials[:, tb * 128:(tb + 1) * 128],
                       selc_sb, True, True)
                nc.scalar.activation(sqcol[:], pscol[:], AF.Sqrt,
                                     bias=eps_col[:])
                nc.vector.reciprocal(rckv_col[:], sqcol[:])

                # broadcast rsqrt factors to 128 partitions, per 512-tok chunk
                for tt in range(4):
                    sl = slice(tt * 512, (tt + 1) * 512)
                    for ty, rct, dst in ((0, rc_hq_t, bc_hq),
                                         (1, rc_kv_t, bc_kv)):
                        psb = pbc.tile([128, 512], f32, tag="pbc")
                        mm(psb, ones_row, rct[:, sl], True, True)
                        bt = bcp.tile([128, 512], f32, name=f"bc{ty}_{tt}")
                        nc.vector.tensor_copy(bt[:], psb[:])
                        dst.append(bt)

            # projections with normalization fused into the PSUM->SBUF step
            with tc.tile_pool(name="p1ps", bufs=3, space="PSUM") as p1ps, \
                 tc.tile_pool(name="p1psv", bufs=2, space="PSUM") as p1psv:
                for tt in range(4):
                    sl = slice(tt * 512, (tt + 1) * 512)
                    for m in range(3):
                        ps = p1ps.tile([128, 512], f32, tag="p1ps")
                        for cc in range(12):
                            mm(ps, wqb_sb[:, cc, m * 128:(m + 1) * 128],
                               hq_all[:, cc, sl], cc == 0, cc == 11)
                        nc.vector.tensor_tensor(qdst[m][:, sl], ps[:],
                                                bc_hq[tt][:], OP.mult)
                    for kh in range(2):
                        ps = p1ps.tile([128, 512], f32, tag="p1ps")
                        for cc in range(4):
                            mm(ps, wkvb_sb[:, cc, kh * 128:(kh + 1) * 128],
                               kv_all[:, cc, sl], cc == 0, cc == 3)
                        nc.vector.tensor_tensor(kn[kh][:, sl], ps[:],
                                                bc_kv[tt][:], OP.mult)
                for tb in range(16):
                    ps = p1psv.tile([128, 256], f32, tag="p1psv")
                    for cc in range(4):
                        mm(ps, kv_all[:, cc, tb * 128:(tb + 1) * 128],
                           wkvb_sb[:, cc, 256:512], cc == 0, cc == 3)
                        # lhsT = latent chunk [c,t], rhs = v cols of wkv_b'^T
                    nc.vector.tensor_scalar_mul(vt[tb][:], ps[:],
                                                rckv_col[:, tb:tb + 1])

                # rope on q (both heads share qt1: rows 0:64 h0, 64:128 h1)
                tmp = prj.tile([128, S], b16)
                for b in (0, 64):
                    nc.sync.dma_start(tmp[b:b + 32], qt1[b + 32:b + 64])
                    nc.sync.dma_start(tmp[b + 32:b + 64], qt1[b:b + 32])
                nc.vector.tensor_tensor(qt1r[:], qt1[:], cos2_sb[:], OP.mult)
                nc.vector.tensor_tensor(tmp[:], tmp[:], sin2_sb[:], OP.mult)
                nc.vector.tensor_tensor(qt1r[:], qt1r[:], tmp[:], OP.add)
                # h1 rope rows to base-0 tile (rows 64: zero; kpe rows 64: 0)
                nc.sync.dma_start(qr1[0:64, :], qt1r[64:128])

        # wo weights prefetch (overlaps attention)
        nc.sync.dma_start(wot_sb[:], woT.ap().rearrange("o p s -> p o s"))

        # attention + split AllGather + column-parallel wo
        with tc.tile_pool(name="apss", bufs=4, space="PSUM") as apss, \
             tc.tile_pool(name="apsx", bufs=2, space="PSUM") as apsx, \
             tc.tile_pool(name="apsd", bufs=1, space="PSUM") as apsd, \
             tc.tile_pool(name="wops", bufs=1, space="PSUM") as wops, \
             tc.tile_pool(name="aex", bufs=4) as aexp, \
             tc.tile_pool(name="asm", bufs=2) as asmp, \
             tc.tile_pool(name="amk", bufs=2) as amkp, \
             tc.tile_pool(name="wop", bufs=1) as wop, \
             tc.tile_pool(name="woot", bufs=3) as wootp:

            LOOKAHEAD = 3

            def attend(h, xh, ship=None):
                qn_h = qn0 if h == 0 else qn1
                qr_h = qt1r if h == 0 else qr1
                for sb in range(4):
                    sl = slice(sb * 512, (sb + 1) * 512)
                    psx = apsx.tile([128, 512], f32, tag="apsx")
                    psdt = apsd.tile([128, 512], f32, tag="dn")
                    psd = psdt[0:1, :]
                    exq = []

                    def consume(tb, ex):
                        mm(psx, vt[tb][:, h * 128:(h + 1) * 128], ex,
                           tb == 0, tb == 15)
                        mm(psd, ones_col, ex, tb == 0, tb == 15)

                    for tb in range(16):
                        tsl = slice(tb * 128, (tb + 1) * 128)
                        pss = apss.tile([128, 512], f32, tag="apss")
                        mm(pss, kn[h][:, tsl], qn_h[:, sl], True, False)
                        mm(pss, kpe_sb[:, tsl], qr_h[:, sl], False, True)
                        if has_mask:
                            mk = amkp.tile([128, 512], b16, tag="amk")
                            nc.sync.dma_start(
                                mk[:], maskT.ap()[tsl, sl])
                            nc.vector.tensor_tensor(pss[:], pss[:], mk[:],
                                                    OP.add)
                        ex = aexp.tile([128, 512], b16, tag="aex")
                        nc.scalar.activation(ex[:], pss[:], AF.Exp)
                        exq.append((tb, ex))
                        if len(exq) > LOOKAHEAD:
                            consume(*exq.pop(0))
                    for item in exq:
                        consume(*item)
                    rd = asmp.tile([1, 512], f32, tag="rd")
                    nc.vector.reciprocal(rd[:], psd[:])
                    psb2 = apsd.tile([128, 512], f32, tag="dn")
                    mm(psb2, ones_row, rd, True, True)
                    rdb = asmp.tile([128, 512], f32, tag="rdb")
                    nc.vector.tensor_copy(rdb[:], psb2[:])
                    nc.vector.tensor_tensor(xh[:, sl], psx[:], rdb[:],
                                            OP.mult)
                    if ship is not None:
                        ship(sb)

            attend(0, xh0)
            nc.sync.dma_start(bounce2a.ap(), xh0[:])
            nc.gpsimd.collective_compute(
                "AllGather", OP.bypass, replica_groups=RG,
                ins=[bounce2a.ap().opt()], outs=[gath2a.ap().opt()])

            # head-1 attention runs while gath2a lands + wo half-accumulates
            def ship_h1(sb):
                if sb in (2, 3):
                    i = sb - 2
                    hs = slice(0, B2SPLIT) if i == 0 else slice(B2SPLIT, S)
                    nc.sync.dma_start(bounce2b[i].ap(), xh1[:, hs])
                    nc.gpsimd.collective_compute(
                        "AllGather", OP.bypass, replica_groups=RG,
                        ins=[bounce2b[i].ap().opt()],
                        outs=[gath2b[i].ap().opt()])

            attend(1, xh1, ship=ship_h1)

            xe_a = wop.tile([128, 8, S], b16)
            for r in range(NCORES):
                nc.sync.dma_start(xe_a[:, r, :], gath2a.ap()[r])
            opart = [wop.tile([128, SSH], f32, name=f"op{st}")
                     for st in range(16)]
            for st in range(16):
                pso = wops.tile([128, SSH], f32, tag="wops")
                for r in range(NCORES):
                    mm(pso, xe_a[:, r, st * 128:(st + 1) * 128],
                       wot_sb[:, 2 * r, :], r == 0, r == 7)
                nc.vector.tensor_copy(opart[st][:], pso[:])

            xe_b = wop.tile([128, 8, S], b16)
            for i in range(2):
                hs = slice(0, B2SPLIT) if i == 0 else slice(B2SPLIT, S)
                for r in range(NCORES):
                    nc.sync.dma_start(xe_b[:, r, hs], gath2b[i].ap()[r])
                for st in (range(12) if i == 0 else range(12, 16)):
                    pso = wops.tile([128, SSH], f32, tag="wops")
                    for r in range(NCORES):
                        mm(pso, xe_b[:, r, st * 128:(st + 1) * 128],
                           wot_sb[:, 2 * r + 1, :], r == 0, r == 7)
                    ot = wootp.tile([128, SSH], f32, tag="ot")
                    nc.vector.tensor_tensor(ot[:], pso[:], opart[st][:],
                                            OP.add)
                    nc.sync.dma_start(out.ap()[st * 128:(st + 1) * 128, :],
                                      ot[:])

    nc.compile()
    return nc


def _prep_inputs(hidden_states, cos, sin, attn_mask, wq_a, q_norm_w, wq_b,
                 wkv_a, kv_norm_w, wkv_b, wo, has_mask):
    import ml_dtypes
    bf = ml_dtypes.bfloat16

    def c(x):
        return np.ascontiguousarray(x.astype(bf))

    hid = np.asarray(hidden_states, np.float32)[0]          # [S, H]
    hidT = c(hid.T.reshape(16, 128, S))                     # [H, S]
    A_T = np.vstack([np.asarray(wq_a, np.float32),
                     np.asarray(wkv_a, np.float32)]).T      # [H, CTOT]
    a_pe = c(A_T[:, 2048:2112].reshape(16, 128, 64))

    cosT = np.asarray(cos, np.float32).T                    # [64, S]
    sinT = np.asarray(sin, np.float32).T
    sinTs = sinT.copy()
    sinTs[0:32] *= -1.0
    cosT2 = c(np.concatenate([cosT, cosT], 0))              # [128, S]
    sinT2s = c(np.concatenate([sinTs, sinTs], 0))

    wqb = np.asarray(wq_b, np.float32) * np.asarray(q_norm_w, np.float32)[None]
    wqb = wqb * SOFTMAX_SCALE
    wkvb = (np.asarray(wkv_b, np.float32)
            * np.asarray(kv_norm_w, np.float32)[None])
    woT_full = np.asarray(wo, np.float32).T                 # [NH*DV, H]

    qperm = np.r_[0:128, 128:192, 320:384, 192:320]
    kvperm = np.r_[0:128, 256:384, 128:256, 384:512]

    selT = np.zeros((16, 2), np.float32)
    for p in range(0, 16, 2):
        if p < 12:
            selT[p, 0] = 1.0 / Q_LORA
        else:
            selT[p, 1] = 1.0 / KV_LORA
    selH = np.ascontiguousarray(selT[:, 0:1])
    selC = np.ascontiguousarray(selT[:, 1:2])

    in_maps = []
    for r in range(NCORES):
        m = {
            "hidT": hidT,
            "a_own": c(A_T[:, r * 256:(r + 1) * 256].reshape(16, 128, 256)),
            "a_pe": a_pe,
            "cosT2": cosT2,
            "sinT2s": sinT2s,
            "wqbT": c(wqb[r * 384:(r + 1) * 384].T[:, qperm]
                      .reshape(12, 128, 384)),
            "wkvbT": c(wkvb[r * 512:(r + 1) * 512].T[:, kvperm]
                       .reshape(4, 128, 512)),
            "woT": c(woT_full[:, r * SSH:(r + 1) * SSH].reshape(16, 128, SSH)),
            "ones_a": np.ones((128, 1), bf),
            "ones_bf": np.ones((1, 128), np.float32),
            "selH": c(selH),
            "selC": c(selC),
        }
        if has_mask:
            m["maskT"] = c(np.asarray(attn_mask, np.float32).T)
        in_maps.append(m)
    return in_maps


def kernel(**inputs):
    from concourse.bass_utils import run_bass_kernel_spmd

    has_mask = bool(np.any(np.asarray(inputs["attn_mask"])))
    if has_mask not in _CACHE:
        _CACHE[has_mask] = _build(has_mask)
    nc = _CACHE[has_mask]

    in_maps = _prep_inputs(has_mask=has_mask, **inputs)
    res = run_bass_kernel_spmd(nc, in_maps, list(range(NCORES))).results
    full = np.concatenate([res[r]["out"] for r in range(NCORES)], axis=1)
    return full.reshape(B, S, H).astype(np.float32)


# revision 23
# speedup vs baseline: 1.6388x; 1.0652x over previous
"""MLA (DeepSeek-style multi-head latent attention) Bass kernel for 8 trn2 NeuronCores.

Sharding: tensor-parallel over heads (2 heads/core) for the big projections +
attention. The low-rank A-projections are CHANNEL-sharded (each core computes
256 of the 2048 hq+kv latent channels for all 2048 tokens) so the AllGathered
latents read back as contiguous 4KB runs; k_pe (64 rope channels) is computed
redundantly on every core, skipping it in the collective. RMS normalization
happens after the gather: per-core partial sum-of-squares rides the AllGather
as one extra bf16 row per channel block, and the rsqrt factors are folded into
the projection PSUM->SBUF copies (free-dim broadcast for q/k, per-partition
tensor_scalar for v). The output projection is column-parallel with a split
AllGather: head 0's attention output gathers and partially accumulates into
wo while head 1's attention still runs.

All tensors bf16 (fp32 PSUM accumulation); exp softmax without max-subtract.
Host-side (free) preprocessing: bf16 casts, weight transposes/permutations,
folding q_norm_w/kv_norm_w and SOFTMAX_SCALE into wq_b/wkv_b, rope sign folds.
"""

import math
import sys

import numpy as np

for _p in ("/opt/trn_rl_repo", "/root/.axon_site/_ro/trn_rl_repo"):
    if _p not in sys.path:
        sys.path.append(_p)

B, S, H = 1, 2048, 2048
NH = 16
Q_LORA, KV_LORA = 1536, 512
D_NOPE, D_ROPE, D_V = 128, 64, 128
D_QK = D_NOPE + D_ROPE
ROPE_FACTOR, MSCALE = 4.0, 1.0
SOFTMAX_SCALE = D_QK ** -0.5 * (0.1 * MSCALE * math.log(ROPE_FACTOR) + 1.0) ** 2
EPS = 1e-6

NCORES = 8
SSH = S // NCORES          # 256 output channels per core (wo column-parallel)
CTOT = Q_LORA + KV_LORA + D_ROPE   # 2112 latent channels

_CACHE = {}


def _build(has_mask: bool):
    import concourse.bacc as bacc
    import concourse.mybir as mybir
    import concourse.tile as tile

    f32 = mybir.dt.float32
    b16 = mybir.dt.bfloat16
    AF = mybir.ActivationFunctionType
    OP = mybir.AluOpType

    nc = bacc.Bacc("TRN2", target_bir_lowering=False, debug=False,
                   num_devices=NCORES)

    hidT = nc.dram_tensor("hidT", [16, 128, S], b16, kind="ExternalInput")
    a_own = nc.dram_tensor("a_own", [16, 128, 256], b16, kind="ExternalInput")
    a_pe = nc.dram_tensor("a_pe", [16, 128, 64], b16, kind="ExternalInput")
    cosT2 = nc.dram_tensor("cosT2", [128, S], b16, kind="ExternalInput")
    sinT2s = nc.dram_tensor("sinT2s", [128, S], b16, kind="ExternalInput")
    wqbT = nc.dram_tensor("wqbT", [12, 128, 384], b16, kind="ExternalInput")
    wkvbT = nc.dram_tensor("wkvbT", [4, 128, 512], b16, kind="ExternalInput")
    woT = nc.dram_tensor("woT", [16, 128, SSH], b16, kind="ExternalInput")
    ones_a = nc.dram_tensor("ones_a", [128, 1], b16, kind="ExternalInput")
    ones_bf = nc.dram_tensor("ones_bf", [1, 128], f32, kind="ExternalInput")
    selH_d = nc.dram_tensor("selH", [16, 1], b16, kind="ExternalInput")
    selC_d = nc.dram_tensor("selC", [16, 1], b16, kind="ExternalInput")
    if has_mask:
        maskT = nc.dram_tensor("maskT", [S, S], b16, kind="ExternalInput")
    out = nc.dram_tensor("out", [S, SSH], f32, kind="ExternalOutput")

    bounce1 = nc.dram_tensor("bounce1", [2, 129, S], b16)
    gath1 = nc.dram_tensor("gath1", [NCORES, 2, 129, S], b16,
                           addr_space="Shared")
    bounce2a = nc.dram_tensor("bounce2a", [128, S], b16)
    gath2a = nc.dram_tensor("gath2a", [NCORES, 128, S], b16,
                            addr_space="Shared")
    B2SPLIT = 1536
    bounce2b = [nc.dram_tensor("bounce2b0", [128, B2SPLIT], b16),
                nc.dram_tensor("bounce2b1", [128, S - B2SPLIT], b16)]
    gath2b = [nc.dram_tensor("gath2b0", [NCORES, 128, B2SPLIT], b16,
                             addr_space="Shared"),
              nc.dram_tensor("gath2b1", [NCORES, 128, S - B2SPLIT], b16,
                             addr_space="Shared")]

    RG = [list(range(NCORES))]

    def mm(ps, lhsT, rhs, start, stop):
        nc.tensor.matmul(ps, lhsT, rhs, start=start, stop=stop)

    from contextlib import ExitStack
    with tile.TileContext(nc) as tc, ExitStack() as _st:
        constp = _st.enter_context(tc.tile_pool(name="const", bufs=1))
        ones_col = constp.tile([128, 1], b16)
        nc.sync.dma_start(ones_col[:], ones_a.ap())
        ones_row = constp.tile([1, 128], f32)
        nc.sync.dma_start(ones_row[:], ones_bf.ap())
        selh_sb = constp.tile([16, 1], b16)
        nc.sync.dma_start(selh_sb[:], selH_d.ap())
        selc_sb = constp.tile([16, 1], b16)
        nc.sync.dma_start(selc_sb[:], selC_d.ap())
        eps1 = constp.tile([1, 1], f32)
        nc.any.memset(eps1[:], EPS)
        eps_col = constp.tile([128, 1], f32)
        nc.any.memset(eps_col[:], EPS)

        # stage-1 weight tiles (DMAs issued after stage-0's loads)
        s1wp = _st.enter_context(tc.tile_pool(name="s1w", bufs=1))
        wqb_sb = s1wp.tile([128, 12, 384], b16)
        wkvb_sb = s1wp.tile([128, 4, 512], b16)
        cos2_sb = s1wp.tile([128, S], b16)
        sin2_sb = s1wp.tile([128, S], b16)
        wot_sb = s1wp.tile([128, 16, SSH], b16)   # DMA issued before attention

        # attention-lifetime pool (also covers the wo epilogue)
        attp = _st.enter_context(tc.tile_pool(name="att", bufs=1))
        kpe_sb = attp.tile([128, S], b16)
        nc.any.memset(kpe_sb[64:128, :], 0.0)

        # ---------------- stage 0: own 256 latent channels for all tokens
        with tc.tile_pool(name="s0", bufs=1) as s0p, \
             tc.tile_pool(name="s0ps", bufs=3, space="PSUM") as s0ps, \
             tc.tile_pool(name="s0ss", bufs=2, space="PSUM") as s0ssp, \
             tc.tile_pool(name="s0pe", bufs=2, space="PSUM") as s0pe, \
             tc.tile_pool(name="s0sq", bufs=3) as s0sqp:
            a_sb = s0p.tile([128, 16, 256], b16)
            hid_sb = s0p.tile([128, 16, S], b16)
            for g in range(8):
                gs = slice(g * 2, (g + 1) * 2)
                nc.sync.dma_start(
                    a_sb[:, gs, :],
                    a_own.ap()[gs].rearrange("o p c -> p o c"))
                nc.sync.dma_start(
                    hid_sb[:, gs, :],
                    hidT.ap()[gs].rearrange("o p s -> p o s"))
            ape_sb = s0p.tile([128, 16, 64], b16)
            nc.sync.dma_start(ape_sb[:], a_pe.ap().rearrange("o p c -> p o c"))
            # stage-1 weights load behind stage-0's operands
            nc.sync.dma_start(cos2_sb[:], cosT2.ap())
            nc.sync.dma_start(sin2_sb[:], sinT2s.ap())
            nc.sync.dma_start(wqb_sb[:],
                              wqbT.ap().rearrange("o p d -> p o d"))
            nc.sync.dma_start(wkvb_sb[:],
                              wkvbT.ap().rearrange("o p d -> p o d"))

            raw = s0p.tile([128, 2, S], b16)
            kpraw = s0p.tile([64, S], b16)
            ssb16 = s0p.tile([1, S], b16)
            zrow = s0p.tile([1, S], b16)
            nc.any.memset(zrow[:], 0.0)
            for tt in range(4):
                sl = slice(tt * 512, (tt + 1) * 512)
                ssp = s0ssp.tile([1, 512], f32, tag="ss")
                for ct in range(2):
                    ps = s0ps.tile([128, 512], f32, tag="s0ps")
                    for hb in range(16):
                        mm(ps, a_sb[:, hb, ct * 128:(ct + 1) * 128],
                           hid_sb[:, hb, sl], hb == 0, hb == 15)
                    nc.vector.tensor_copy(raw[:, ct, sl], ps[:])
                    sq = s0sqp.tile([128, 512], b16, tag="s0sq")
                    nc.scalar.activation(sq[:], ps[:], AF.Square)
                    mm(ssp, ones_col, sq, ct == 0, ct == 1)
                nc.scalar.copy(ssb16[:, sl], ssp[:])

            # ship bounce pieces as each token chunk's latents finish;
            # the single collective then waits only on the last piece
            for i in range(4):
                hs = slice(i * 512, (i + 1) * 512)
                nc.sync.dma_start(
                    bounce1.ap()[:, 0:128, hs].rearrange("o p s -> p o s"),
                    raw[:, :, hs])
            nc.sync.dma_start(bounce1.ap()[0, 128:129, :], ssb16[:])
            nc.sync.dma_start(bounce1.ap()[1, 128:129, :], zrow[:])
            nc.gpsimd.collective_compute(
                "AllGather", OP.bypass, replica_groups=RG,
                ins=[bounce1.ap().opt()], outs=[gath1.ap().opt()])

            # k_pe matmuls after the ship: they fill the AllGather wait
            for tt in range(4):
                sl = slice(tt * 512, (tt + 1) * 512)
                kp = s0pe.tile([64, 512], f32, tag="kpe")
                for hb in range(16):
                    mm(kp, ape_sb[:, hb, :], hid_sb[:, hb, sl],
                       hb == 0, hb == 15)
                nc.vector.tensor_copy(kpraw[:, sl], kp[:])

            # k_pe rope (redundant on every core; not in the collective)
            t1 = s0p.tile([64, S], b16)
            nc.vector.tensor_tensor(t1[:], kpraw[:], cos2_sb[0:64, :], OP.mult)
            rsw = s0p.tile([64, S], b16)
            nc.sync.dma_start(rsw[0:32], kpraw[32:64])
            nc.sync.dma_start(rsw[32:64], kpraw[0:32])
            nc.vector.tensor_tensor(rsw[:], rsw[:], sin2_sb[0:64, :], OP.mult)
            nc.vector.tensor_tensor(kpe_sb[0:64, :], t1[:], rsw[:], OP.add)

        # ---------------- stage 1: per-head projections + attention
        qn0 = attp.tile([128, S], b16)
        qt1 = attp.tile([128, S], b16)
        qn1 = attp.tile([128, S], b16)
        qdst = (qn0, qt1, qn1)
        kn0 = attp.tile([128, S], b16)
        kn1 = attp.tile([128, S], b16)
        kn = (kn0, kn1)
        vt = [attp.tile([128, 256], b16, name=f"vt{tb}") for tb in range(16)]
        qt1r = attp.tile([128, S], b16)
        qr1 = attp.tile([128, S], b16)
        nc.any.memset(qr1[64:128, :], 0.0)
        xh0 = attp.tile([128, S], b16)
        xh1 = attp.tile([128, S], b16)

        with tc.tile_pool(name="proj", bufs=1) as prj, \
             tc.tile_pool(name="bcp", bufs=1) as bcp:
            # gathered reads: all contiguous 4KB runs
            partials = prj.tile([16, S], b16)
            hq_all = prj.tile([128, 12, S], b16)
            kv_all = prj.tile([128, 4, S], b16)
            nc.sync.dma_start(
                partials[:],
                gath1.ap()[:, :, 128, :].rearrange("a b s -> (a b) s"))
            for tt in range(4):
                sl = slice(tt * 512, (tt + 1) * 512)
                for r in (6, 7):
                    nc.sync.dma_start(
                        kv_all[:, 2 * (r - 6):2 * (r - 6) + 2, sl],
                        gath1.ap()[r, :, 0:128, sl]
                        .rearrange("o p s -> p o s"))
                for r in range(6):
                    nc.sync.dma_start(
                        hq_all[:, 2 * r:2 * r + 2, sl],
                        gath1.ap()[r, :, 0:128, sl]
                        .rearrange("o p s -> p o s"))

            # rms factors: [1, S] free-layout (q/k) + [128, 16] col-layout (v)
            sq_hq = prj.tile([1, S], f32)
            sq_kv = prj.tile([1, S], f32)
            rc_hq_t = prj.tile([1, S], f32)
            rc_kv_t = prj.tile([1, S], f32)
            sqcol = prj.tile([128, 16], f32)
            rckv_col = prj.tile([128, 16], f32)
            bc_hq, bc_kv = [], []
            with tc.tile_pool(name="pfac", bufs=2, space="PSUM") as pfac, \
                 tc.tile_pool(name="pbc", bufs=2, space="PSUM") as pbc:
                for tt in range(4):
                    sl = slice(tt * 512, (tt + 1) * 512)
                    for selt, sqt in ((selh_sb, sq_hq), (selc_sb, sq_kv)):
                        ps2 = pfac.tile([1, 512], f32, tag="ps2")
                        mm(ps2, selt, partials[:, sl], True, True)
                        nc.scalar.activation(sqt[:, sl], ps2[:], AF.Sqrt,
                                             bias=eps1[:])
                nc.vector.reciprocal(rc_hq_t[:], sq_hq[:])
                nc.vector.reciprocal(rc_kv_t[:], sq_kv[:])
                pscol = pfac.tile([128, 16], f32, tag="pscol")
                for tb in range(16):
                    mm(pscol[:, tb:tb + 1],
                       partials[:, tb * 128:(tb + 1) * 128],
                       selc_sb, True, True)
                nc.scalar.activation(sqcol[:], pscol[:], AF.Sqrt,
                                     bias=eps_col[:])
                nc.vector.reciprocal(rckv_col[:], sqcol[:])

                # broadcast rsqrt factors to 128 partitions, per 512-tok chunk
                for tt in range(4):
                    sl = slice(tt * 512, (tt + 1) * 512)
                    for ty, rct, dst in ((0, rc_hq_t, bc_hq),
                                         (1, rc_kv_t, bc_kv)):
                        psb = pbc.tile([128, 512], f32, tag="pbc")
                        mm(psb, ones_row, rct[:, sl], True, True)
                        bt = bcp.tile([128, 512], f32, name=f"bc{ty}_{tt}")
                        nc.vector.tensor_copy(bt[:], psb[:])
                        dst.append(bt)

            # projections with normalization fused into the PSUM->SBUF step
            with tc.tile_pool(name="p1ps", bufs=3, space="PSUM") as p1ps, \
                 tc.tile_pool(name="p1psv", bufs=2, space="PSUM") as p1psv:
                for tt in range(4):
                    sl = slice(tt * 512, (tt + 1) * 512)
                    for m in range(3):
                        ps = p1ps.tile([128, 512], f32, tag="p1ps")
                        for cc in range(12):
                            mm(ps, wqb_sb[:, cc, m * 128:(m + 1) * 128],
                               hq_all[:, cc, sl], cc == 0, cc == 11)
                        nc.vector.tensor_tensor(qdst[m][:, sl], ps[:],
                                                bc_hq[tt][:], OP.mult)
                    for kh in range(2):
                        ps = p1ps.tile([128, 512], f32, tag="p1ps")
                        for cc in range(4):
                            mm(ps, wkvb_sb[:, cc, kh * 128:(kh + 1) * 128],
                               kv_all[:, cc, sl], cc == 0, cc == 3)
                        nc.vector.tensor_tensor(kn[kh][:, sl], ps[:],
                                                bc_kv[tt][:], OP.mult)
                for tb in range(16):
                    ps = p1psv.tile([128, 256], f32, tag="p1psv")
                    for cc in range(4):
                        mm(ps, kv_all[:, cc, tb * 128:(tb + 1) * 128],
                           wkvb_sb[:, cc, 256:512], cc == 0, cc == 3)
                        # lhsT = latent chunk [c,t], rhs = v cols of wkv_b'^T
                    nc.vector.tensor_scalar_mul(vt[tb][:], ps[:],
                                                rckv_col[:, tb:tb + 1])

                # rope on q (both heads share qt1: rows 0:64 h0, 64:128 h1)
                tmp = prj.tile([128, S], b16)
                for b in (0, 64):
                    nc.sync.dma_start(tmp[b:b + 32], qt1[b + 32:b + 64])
                    nc.sync.dma_start(tmp[b + 32:b + 64], qt1[b:b + 32])
                nc.vector.tensor_tensor(qt1r[:], qt1[:], cos2_sb[:], OP.mult)
                nc.vector.tensor_tensor(tmp[:], tmp[:], sin2_sb[:], OP.mult)
                nc.vector.tensor_tensor(qt1r[:], qt1r[:], tmp[:], OP.add)
                # h1 rope rows to base-0 tile (rows 64: zero; kpe rows 64: 0)
                nc.sync.dma_start(qr1[0:64, :], qt1r[64:128])

        # wo weights prefetch (overlaps attention)
        nc.sync.dma_start(wot_sb[:], woT.ap().rearrange("o p s -> p o s"))

        # attention + split AllGather + column-parallel wo
        with tc.tile_pool(name="apss", bufs=3, space="PSUM") as apss, \
             tc.tile_pool(name="apsx", bufs=2, space="PSUM") as apsx, \
             tc.tile_pool(name="apsd", bufs=2, space="PSUM") as apsd, \
             tc.tile_pool(name="wops", bufs=1, space="PSUM") as wops, \
             tc.tile_pool(name="aex", bufs=4) as aexp, \
             tc.tile_pool(name="asm", bufs=2) as asmp, \
             tc.tile_pool(name="amk", bufs=2) as amkp, \
             tc.tile_pool(name="wop", bufs=1) as wop, \
             tc.tile_pool(name="woot", bufs=3) as wootp:

            LOOKAHEAD = 3

            def attend(h, xh, ship=None):
                qn_h = qn0 if h == 0 else qn1
                qr_h = qt1r if h == 0 else qr1

                def finish(sb, psx, psdt):
                    # softmax normalize for a finished chunk; emitted a few
                    # matmuls into the NEXT chunk so the in-order PE never
                    # stalls on the (slow, single-lane) reciprocal
                    sl = slice(sb * 512, (sb + 1) * 512)
                    rd = asmp.tile([1, 512], f32, tag="rd")
                    nc.vector.reciprocal(rd[:], psdt[0:1, :])
                    psb2 = apsd.tile([128, 512], f32, tag="dn")
                    mm(psb2, ones_row, rd, True, True)
                    rdb = asmp.tile([128, 512], f32, tag="rdb")
                    nc.vector.tensor_copy(rdb[:], psb2[:])
                    nc.vector.tensor_tensor(xh[:, sl], psx[:], rdb[:],
                                            OP.mult)
                    if ship is not None:
                        ship(sb)

                pending = None
                for sb in range(4):
                    sl = slice(sb * 512, (sb + 1) * 512)
                    psx = apsx.tile([128, 512], f32, tag="apsx")
                    psdt = apsd.tile([128, 512], f32, tag="dn")
                    psd = psdt[0:1, :]
                    exq = []

                    def consume(tb, ex):
                        mm(psx, vt[tb][:, h * 128:(h + 1) * 128], ex,
                           tb == 0, tb == 15)
                        mm(psd, ones_col, ex, tb == 0, tb == 15)

                    for tb in range(16):
                        tsl = slice(tb * 128, (tb + 1) * 128)
                        pss = apss.tile([128, 512], f32, tag="apss")
                        mm(pss, kn[h][:, tsl], qn_h[:, sl], True, False)
                        mm(pss, kpe_sb[:, tsl], qr_h[:, sl], False, True)
                        if has_mask:
                            mk = amkp.tile([128, 512], b16, tag="amk")
                            nc.sync.dma_start(
                                mk[:], maskT.ap()[tsl, sl])
                            nc.vector.tensor_tensor(pss[:], pss[:], mk[:],
                                                    OP.add)
                        ex = aexp.tile([128, 512], b16, tag="aex")
                        nc.scalar.activation(ex[:], pss[:], AF.Exp)
                        exq.append((tb, ex))
                        if tb == 4 and pending is not None:
                            finish(*pending)
                            pending = None
                        if len(exq) > LOOKAHEAD:
                            consume(*exq.pop(0))
                    for item in exq:
                        consume(*item)
                    pending = (sb, psx, psdt)
                finish(*pending)

            attend(0, xh0)
            nc.sync.dma_start(bounce2a.ap(), xh0[:])
            nc.gpsimd.collective_compute(
                "AllGather", OP.bypass, replica_groups=RG,
                ins=[bounce2a.ap().opt()], outs=[gath2a.ap().opt()])

            # head-1 attention runs while gath2a lands + wo half-accumulates
            def ship_h1(sb):
                if sb in (2, 3):
                    i = sb - 2
                    hs = slice(0, B2SPLIT) if i == 0 else slice(B2SPLIT, S)
                    nc.sync.dma_start(bounce2b[i].ap(), xh1[:, hs])
                    nc.gpsimd.collective_compute(
                        "AllGather", OP.bypass, replica_groups=RG,
                        ins=[bounce2b[i].ap().opt()],
                        outs=[gath2b[i].ap().opt()])

            attend(1, xh1, ship=ship_h1)

            xe_a = wop.tile([128, 8, S], b16)
            for r in range(NCORES):
                nc.sync.dma_start(xe_a[:, r, :], gath2a.ap()[r])
            opart = [wop.tile([128, SSH], f32, name=f"op{st}")
                     for st in range(16)]
            for st in range(16):
                pso = wops.tile([128, SSH], f32, tag="wops")
                for r in range(NCORES):
                    mm(pso, xe_a[:, r, st * 128:(st + 1) * 128],
                       wot_sb[:, 2 * r, :], r == 0, r == 7)
                nc.vector.tensor_copy(opart[st][:], pso[:])

            xe_b = wop.tile([128, 8, S], b16)
            for i in range(2):
                hs = slice(0, B2SPLIT) if i == 0 else slice(B2SPLIT, S)
                for r in range(NCORES):
                    nc.sync.dma_start(xe_b[:, r, hs], gath2b[i].ap()[r])
                for st in (range(12) if i == 0 else range(12, 16)):
                    pso = wops.tile([128, SSH], f32, tag="wops")
                    for r in range(NCORES):
                        mm(pso, xe_b[:, r, st * 128:(st + 1) * 128],
                           wot_sb[:, 2 * r + 1, :], r == 0, r == 7)
                    ot = wootp.tile([128, SSH], f32, tag="ot")
                    nc.vector.tensor_tensor(ot[:], pso[:], opart[st][:],
                                            OP.add)
                    nc.sync.dma_start(out.ap()[st * 128:(st + 1) * 128, :],
                                      ot[:])

    nc.compile()
    return nc


def _prep_inputs(hidden_states, cos, sin, attn_mask, wq_a, q_norm_w, wq_b,
                 wkv_a, kv_norm_w, wkv_b, wo, has_mask):
    import ml_dtypes
    bf = ml_dtypes.bfloat16

    def c(x):
        return np.ascontiguousarray(x.astype(bf))

    hid = np.asarray(hidden_states, np.float32)[0]          # [S, H]
    hidT = c(hid.T.reshape(16, 128, S))                     # [H, S]
    A_T = np.vstack([np.asarray(wq_a, np.float32),
                     np.asarray(wkv_a, np.float32)]).T      # [H, CTOT]
    a_pe = c(A_T[:, 2048:2112].reshape(16, 128, 64))

    cosT = np.asarray(cos, np.float32).T                    # [64, S]
    sinT = np.asarray(sin, np.float32).T
    sinTs = sinT.copy()
    sinTs[0:32] *= -1.0
    cosT2 = c(np.concatenate([cosT, cosT], 0))              # [128, S]
    sinT2s = c(np.concatenate([sinTs, sinTs], 0))

    wqb = np.asarray(wq_b, np.float32) * np.asarray(q_norm_w, np.float32)[None]
    wqb = wqb * SOFTMAX_SCALE
    wkvb = (np.asarray(wkv_b, np.float32)
            * np.asarray(kv_norm_w, np.float32)[None])
    woT_full = np.asarray(wo, np.float32).T                 # [NH*DV, H]

    qperm = np.r_[0:128, 128:192, 320:384, 192:320]
    kvperm = np.r_[0:128, 256:384, 128:256, 384:512]

    selT = np.zeros((16, 2), np.float32)
    for p in range(0, 16, 2):
        if p < 12:
            selT[p, 0] = 1.0 / Q_LORA
        else:
            selT[p, 1] = 1.0 / KV_LORA
    selH = np.ascontiguousarray(selT[:, 0:1])
    selC = np.ascontiguousarray(selT[:, 1:2])

    in_maps = []
    for r in range(NCORES):
        m = {
            "hidT": hidT,
            "a_own": c(A_T[:, r * 256:(r + 1) * 256].reshape(16, 128, 256)),
            "a_pe": a_pe,
            "cosT2": cosT2,
            "sinT2s": sinT2s,
            "wqbT": c(wqb[r * 384:(r + 1) * 384].T[:, qperm]
                      .reshape(12, 128, 384)),
            "wkvbT": c(wkvb[r * 512:(r + 1) * 512].T[:, kvperm]
                       .reshape(4, 128, 512)),
            "woT": c(woT_full[:, r * SSH:(r + 1) * SSH].reshape(16, 128, SSH)),
            "ones_a": np.ones((128, 1), bf),
            "ones_bf": np.ones((1, 128), np.float32),
            "selH": c(selH),
            "selC": c(selC),
        }
        if has_mask:
            m["maskT"] = c(np.asarray(attn_mask, np.float32).T)
        in_maps.append(m)
    return in_maps


def kernel(**inputs):
    from concourse.bass_utils import run_bass_kernel_spmd

    has_mask = bool(np.any(np.asarray(inputs["attn_mask"])))
    if has_mask not in _CACHE:
        _CACHE[has_mask] = _build(has_mask)
    nc = _CACHE[has_mask]

    in_maps = _prep_inputs(has_mask=has_mask, **inputs)
    res = run_bass_kernel_spmd(nc, in_maps, list(range(NCORES))).results
    full = np.concatenate([res[r]["out"] for r in range(NCORES)], axis=1)
    return full.reshape(B, S, H).astype(np.float32)
